# revision 17
# baseline (speedup 1.0000x reference)
"""Trainium2 Bass kernel for nn_BiModel (2-layer bidirectional GCN).

Distribution over 8 NeuronCores: nodes sharded 6250/core, edge lists
partitioned by destination core and sorted by (branch, dst-block,
src-half).  Per GCN layer each core computes the transformed features
for its own nodes (prescaled by dinv[src]), AllGathers the bf16
message table, dma_gathers the src rows of its edges and segment-sums
them with one-hot matmuls on the tensor engine (PSUM accumulation per
128-node destination block).  Host work is limited to sharding/layout
prep (transpose/pad, edge partition+sort, degree counts, gather index
tensors); all floating-point math on tensor data runs on device.

v2 perf changes vs baseline:
- exact data-driven chunk capacities (no floors): ~16% fewer gather
  descriptors / masks / matmuls
- dma_gather descriptor generation round-robins 4 SWDGE queues (2-way
  parallel Q7 generation measured on HW)
- one-hot masks built in ONE broadcast-AP tensor_tensor per
  (branch-pair, supergroup) instead of one DVE op per chunk
- gathers read the AllGather output directly (no DRAM->DRAM copy)
- message buffers double-buffered across supergroups
"""

import contextlib
import ctypes
import glob
import json
import os
import subprocess
import sys
import tempfile
import types

import numpy as np

import concourse.bass as bass
import concourse.bacc as bacc
import concourse.mybir as mybir
import concourse.tile as tile
from concourse.bass_utils import run_bass_kernel_spmd
from concourse.masks import make_identity

import ml_dtypes

P = 128
F32 = mybir.dt.float32
BF16 = mybir.dt.bfloat16
I16 = mybir.dt.int16
I32 = mybir.dt.int32
HALF = 32768

FULL_CFG = dict(n=50000, e=800000, f_in=500, h=64, c_out=16, n_cores=8,
                sb_blocks=4, cap_floor=None, nqueues=4)


def cdiv(a, b):
    return (a + b - 1) // b


# ----------------------------------------------------------------------------
# host-side layout / preprocessing
# ----------------------------------------------------------------------------

class Layout:
    """Compile-time chunk layout, shared by all cores (uniform SPMD
    program).  Group = (branch, dst-block, src-half); per-group chunk
    capacity = max edge count over cores, rounded up to 128."""

    def __init__(self, cfg, group_counts):
        # group_counts: [n_cores, 2, nblk, 2]
        self.cfg = cfg
        self.nloc = cfg["n"] // cfg["n_cores"]
        self.nblk = cdiv(self.nloc, P)
        cap = group_counts.max(axis=0)                    # [2, nblk, 2]
        self.cap_chunks = cdiv(cap, P)                    # may be 0
        floor = cfg.get("cap_floor")
        if floor is not None:
            flo, fhi = floor
            self.cap_chunks = np.maximum(
                self.cap_chunks,
                np.array([[[flo, fhi]]], np.int64))
        # branch-major chunk stream (for dstloc): (blk, half) order
        self.chunk_off = np.zeros((2, self.nblk, 2), np.int64)
        for b in range(2):
            off = 0
            for blk in range(self.nblk):
                for hf in range(2):
                    self.chunk_off[b, blk, hf] = off
                    off += self.cap_chunks[b, blk, hf]
        self.nchunks_br = self.chunk_off[:, -1, 1] + self.cap_chunks[:, -1, 1]
        # per-(branch, half) gather stream: blk order
        self.half_chunk_off = np.zeros((2, self.nblk, 2), np.int64)
        self.nchunks_bh = np.zeros((2, 2), np.int64)
        for b in range(2):
            for hf in range(2):
                off = 0
                for blk in range(self.nblk):
                    self.half_chunk_off[b, blk, hf] = off
                    off += self.cap_chunks[b, blk, hf]
                self.nchunks_bh[b, hf] = off
        sb = cfg["sb_blocks"]
        self.sg_blocks = [list(range(g * sb, min((g + 1) * sb, self.nblk)))
                          for g in range(cdiv(self.nblk, sb))]

    def signature(self):
        return (tuple(self.cap_chunks.reshape(-1).tolist()),
                tuple(sorted((k, str(v)) for k, v in self.cfg.items())))


def _wrap_idx16(idx, n_pad):
    buf = np.zeros(n_pad, np.int16)
    buf[: len(idx)] = idx.astype(np.int16)
    w = buf.reshape(n_pad // 16, 16).T            # [16, n/16]
    return np.ascontiguousarray(np.tile(w, (8, 1)))  # [128, n/16]


def host_prep(cfg, x, edge_index, is_reversed):
    n, f_in = cfg["n"], cfg["f_in"]
    n_cores = cfg["n_cores"]
    nloc = n // n_cores
    f_pad = cdiv(f_in, P) * P

    src = np.asarray(edge_index[0], np.int64)
    dst = np.asarray(edge_index[1], np.int64)
    rev = np.asarray(is_reversed).astype(bool)

    core = dst // nloc
    dl = dst % nloc
    blk = dl // P
    sblk = (dl % P).astype(np.float32)
    branch = rev.astype(np.int64)
    hf = (src >= HALF).astype(np.int64)

    nblk = cdiv(nloc, P)
    key = ((core * 2 + branch) * nblk + blk) * 2 + hf
    order = np.argsort(key, kind="stable")
    counts = np.bincount(key[order], minlength=n_cores * 2 * nblk * 2)
    counts = counts.reshape(n_cores, 2, nblk, 2)
    lay = Layout(cfg, counts)

    deg = np.zeros((2, n), np.float32)
    np.add.at(deg[0], dst[~rev], 1.0)
    np.add.at(deg[1], dst[rev], 1.0)

    xT = np.zeros((f_pad, n), ml_dtypes.bfloat16)
    xT[:f_in] = np.asarray(x, np.float32).T
    kch = f_pad // P

    src_s = src[order]
    sblk_s = sblk[order]
    gs = np.concatenate([[0], np.cumsum(counts.reshape(-1))])[:-1]
    gs = gs.reshape(n_cores, 2, nblk, 2)

    nblk_pad = nblk * P
    in_maps = []
    for c in range(n_cores):
        xc = xT[:, c * nloc:(c + 1) * nloc].reshape(kch, P, nloc)
        m = {"xT": np.ascontiguousarray(
            xc.transpose(1, 0, 2).reshape(P, kch * nloc))}
        degs = np.ones((P, 2 * nblk), np.float32)
        for b in range(2):
            dloc = np.ones(nblk_pad, np.float32)
            dloc[:nloc] = deg[b, c * nloc:(c + 1) * nloc]
            degs[:, b * nblk:(b + 1) * nblk] = dloc.reshape(nblk, P).T
        m["degs"] = degs
        for b in range(2):
            ncol = max(int(lay.nchunks_br[b]), 1)
            dst_cols = np.full((P, ncol), -1.0, np.float32)
            for hf_ in range(2):
                tot = max(int(lay.nchunks_bh[b, hf_]), 1) * P
                idx_stream = np.zeros(tot, np.int16)
                for blk_ in range(lay.nblk):
                    cnt = int(counts[c, b, blk_, hf_])
                    s0 = int(gs[c, b, blk_, hf_])
                    ho = int(lay.half_chunk_off[b, blk_, hf_]) * P
                    idx_stream[ho:ho + cnt] = src_s[s0:s0 + cnt] - hf_ * HALF
                    co = int(lay.chunk_off[b, blk_, hf_])
                    ce = int(lay.cap_chunks[b, blk_, hf_])
                    if ce:
                        dv = np.full(ce * P, -1.0, np.float32)
                        dv[:cnt] = sblk_s[s0:s0 + cnt]
                        dst_cols[:, co:co + ce] = dv.reshape(ce, P).T
                m[f"idx_b{b}h{hf_}"] = _wrap_idx16(idx_stream, tot)
            m[f"dstloc_b{b}"] = dst_cols.astype(ml_dtypes.bfloat16)
        in_maps.append(m)
    return lay, in_maps


def host_prep_weights(cfg, W_st0, b_st0, W_ts0, b_ts0, W_st1, b_st1,
                      W_ts1, b_ts1, W_last, b_last):
    f_in, h, c_out = cfg["f_in"], cfg["h"], cfg["c_out"]
    f_pad = cdiv(f_in, P) * P
    W0 = np.zeros((f_pad, 2 * h), np.float32)
    W0[:f_in, :h] = W_st0
    W0[:f_in, h:] = W_ts0
    kch = f_pad // P
    W0 = np.ascontiguousarray(
        W0.reshape(kch, P, 2 * h).transpose(1, 0, 2).reshape(P, kch * 2 * h))
    W1 = np.concatenate([W_st1, W_ts1], axis=1).astype(np.float32)
    WL = np.zeros((2 * h, 128), np.float32)
    WL[:, :c_out] = W_last
    bias01 = np.stack([np.concatenate([b_st0, b_ts0]),
                       np.concatenate([b_st1, b_ts1])], axis=1).astype(np.float32)
    return dict(W0=W0.astype(ml_dtypes.bfloat16),
                W1=W1.astype(ml_dtypes.bfloat16),
                WL=WL.astype(ml_dtypes.bfloat16), bias01=bias01,
                b_last=np.asarray(b_last, np.float32).reshape(c_out, 1))


# ----------------------------------------------------------------------------
# device program
# ----------------------------------------------------------------------------

def build_program(cfg, lay, stop_at=99, agg_mode="full"):
    n, f_in = cfg["n"], cfg["f_in"]
    h, c_out = cfg["h"], cfg["c_out"]
    n_cores = cfg["n_cores"]
    nqueues = cfg.get("nqueues", 4)
    nloc = n // n_cores
    nblk = lay.nblk
    nblk_pad = nblk * P
    f_pad = cdiv(f_in, P) * P
    kch = f_pad // P
    h2 = 2 * h
    core_ids = list(range(n_cores))

    nc = bacc.Bacc("TRN2", target_bir_lowering=False, debug=False,
                   num_devices=n_cores, num_swdge_queues=nqueues)

    xT_d = nc.declare_dram_parameter("xT", [P, kch * nloc], BF16, isOutput=False)
    degs_d = nc.declare_dram_parameter("degs", [P, 2 * nblk], F32, isOutput=False)
    W0_d = nc.declare_dram_parameter("W0", [P, kch * h2], BF16, isOutput=False)
    W1_d = nc.declare_dram_parameter("W1", [h2, h2], BF16, isOutput=False)
    WL_d = nc.declare_dram_parameter("WL", [h2, 128], BF16, isOutput=False)
    bias01_d = nc.declare_dram_parameter("bias01", [h2, 2], F32, isOutput=False)
    b_last_d = nc.declare_dram_parameter("b_last", [c_out, 1], F32, isOutput=False)
    idx_d = {}
    for b in range(2):
        for hf in range(2):
            w = max(int(lay.nchunks_bh[b, hf]), 1) * 8
            idx_d[b, hf] = nc.declare_dram_parameter(
                f"idx_b{b}h{hf}", [P, w], I16, isOutput=False)
    dstloc_d = [nc.declare_dram_parameter(
        f"dstloc_b{b}", [P, max(int(lay.nchunks_br[b]), 1)], BF16,
        isOutput=False) for b in range(2)]
    out_d = nc.declare_dram_parameter("out", [nloc, c_out], F32, isOutput=True)

    tbl_loc = [nc.dram_tensor(f"tbl_loc{i}", [nloc, h2], BF16) for i in range(2)]
    tbl_full = [nc.dram_tensor(f"tbl_full{i}", [n, h2], BF16,
                               addr_space="Shared") for i in range(2)]
    nblk_pad_ = cdiv(n // n_cores, P) * P
    dinv_flat_d = nc.dram_tensor("dinv_flat", [1, 3 * nblk_pad_], F32)
    tblL_loc = nc.dram_tensor("tblL_loc", [nloc, 128], BF16)
    tblL_full = nc.dram_tensor("tblL_full", [n, 128], BF16, addr_space="Shared")

    qctr = [0]

    def next_q():
        q = qctr[0] % nqueues
        qctr[0] += 1
        return q

    with tile.TileContext(nc) as tc:
        with (
            tc.tile_pool(name="persist", bufs=1) as pp,
            tc.tile_pool(name="work", bufs=2) as wp,
            tc.tile_pool(name="msg", bufs=2) as mp,
            tc.tile_pool(name="mask", bufs=3) as kp,
            tc.tile_pool(name="psum", bufs=2, space="PSUM") as psp,
        ):
            # ---------- constants ----------
            iota_i = wp.tile([P, P], I32, tag="ioi")
            nc.gpsimd.iota(iota_i[:], pattern=[[1, P]], base=0,
                           channel_multiplier=0)
            iota3 = pp.tile([P, 1, P], BF16, tag="io3")
            nc.vector.tensor_copy(iota3[:, 0, :], iota_i[:])
            ident = pp.tile([P, P], F32, tag="ident")
            make_identity(nc, ident[:])
            biasv = pp.tile([P, 2], F32, tag="biasv")
            nc.sync.dma_start(out=biasv[:], in_=bias01_d[:, :])
            biasL = pp.tile([c_out, 1], F32, tag="biasL")
            nc.sync.dma_start(out=biasL[:], in_=b_last_d[:, :])

            def barrier():
                tc.strict_bb_all_engine_barrier()

            # ---------- degrees -> dinv [128, 3*nblk] (st|ts|all) ----------
            deg_sb = wp.tile([P, 2 * nblk], F32, tag="degsb")
            nc.sync.dma_start(out=deg_sb[:], in_=degs_d[:, :])
            dtmp = wp.tile([P, 3 * nblk], F32, tag="dtmp")
            nc.vector.tensor_tensor(out=dtmp[:, 2 * nblk:],
                                    in0=deg_sb[:, :nblk], in1=deg_sb[:, nblk:],
                                    op=mybir.AluOpType.add)
            nc.vector.tensor_copy(dtmp[:, :2 * nblk], deg_sb[:])
            nc.vector.tensor_scalar_add(dtmp[:], dtmp[:], 1.0)
            dsq = wp.tile([P, 3 * nblk], F32, tag="dsq")
            nc.scalar.sqrt(dsq[:], dtmp[:])
            dinv = pp.tile([P, 3 * nblk], F32, tag="dinv")
            nc.vector.reciprocal(dinv[:], dsq[:])

            # transposed rows: dinvT [nblk, 3*128]
            dinvT = pp.tile([nblk, 3 * P], F32, tag="dinvT")
            for i in range(3):
                tps = psp.tile([nblk, P], F32, tag="pst")
                nc.tensor.transpose(tps[:], dinv[:, i * nblk:(i + 1) * nblk],
                                    ident[:])
                nc.scalar.copy(dinvT[:, i * P:(i + 1) * P], tps[:])

            # flatten dinvT rows into DRAM [1, 3*nblk_pad]
            for i in range(3):
                nc.sync.dma_start(
                    out=dinv_flat_d[0:1, i * nblk_pad:(i + 1) * nblk_pad],
                    in_=dinvT[:, i * P:(i + 1) * P])

            # broadcast tiles via K=1 matmul: ones[1,M].T @ row[1,N]
            ones_row = pp.tile([1, P], F32, tag="ones_row")
            nc.vector.memset(ones_row[:], 1.0)
            dinvb = pp.tile([P, nblk_pad], BF16, tag="dinvb")
            dinvallb = pp.tile([c_out, nblk_pad], BF16, tag="dinvallb")
            NTB = 512
            for t0 in range(0, nblk_pad, NTB):
                t1 = min(t0 + NTB, nblk_pad)
                dfs = wp.tile([1, 3 * NTB], F32, tag="dfs")
                for i in range(3):
                    nc.sync.dma_start(
                        out=dfs[0:1, i * NTB: i * NTB + t1 - t0],
                        in_=dinv_flat_d[0:1, i * nblk_pad + t0: i * nblk_pad + t1])
                bps = psp.tile([P, NTB], F32, tag="pst")
                nc.tensor.matmul(bps[0:h, :t1 - t0], lhsT=ones_row[0:1, 0:h],
                                 rhs=dfs[0:1, 0:t1 - t0],
                                 start=True, stop=True)
                nc.tensor.matmul(bps[h:h2, :t1 - t0], lhsT=ones_row[0:1, 0:h],
                                 rhs=dfs[0:1, NTB:NTB + t1 - t0],
                                 start=True, stop=True, tile_position=(0, h))
                nc.scalar.copy(dinvb[:, t0:t1], bps[:, :t1 - t0])
                bps2 = psp.tile([c_out, NTB], F32, tag="psnm")
                nc.tensor.matmul(bps2[:, :t1 - t0], lhsT=ones_row[0:1, 0:c_out],
                                 rhs=dfs[0:1, 2 * NTB:2 * NTB + t1 - t0],
                                 start=True, stop=True)
                nc.scalar.copy(dinvallb[:, t0:t1], bps2[:, :t1 - t0])

            # ---------- weights ----------
            w0_sb = pp.tile([P, kch * h2], BF16, tag="w0")
            nc.sync.dma_start(out=w0_sb[:], in_=W0_d[:, :])
            w1_sb = pp.tile([P, h2], BF16, tag="w1")
            nc.sync.dma_start(out=w1_sb[:], in_=W1_d[:, :])
            wl_sb = pp.tile([P, 128], BF16, tag="wl")
            nc.sync.dma_start(out=wl_sb[:], in_=WL_d[:, :])

            # ---------- state ----------
            hT = pp.tile([P, nblk_pad], BF16, tag="hT")
            h2T = pp.tile([P, nblk_pad], BF16, tag="h2T")
            xwT = pp.tile([P, nblk_pad], BF16, tag="xwT")
            aggT = pp.tile([P, nblk_pad], F32, tag="aggT")
            # last-layer [16,*] views over tiles that are dead by then
            xwTL = xwT[0:c_out, :]
            outTL = aggT[0:c_out, :]

            # ---------------------------------------------------------------
            def build_tables(src_getter, src_kch, w_cols_of_k, wcols,
                             slp_rows, slp_dst, tbl_dst, prescale):
                """src_getter(k, j0, j1) -> [128, j1-j0] AP of input chunk k;
                w_cols_of_k(k) -> [128, wcols] weight AP.
                Writes transposed xw to slp_dst[:slp_rows] and prescaled
                bf16 node-major rows to tbl_dst."""
                NT = 512
                for t0 in range(0, nloc, NT):
                    t1 = min(t0 + NT, nloc)
                    ps = psp.tile([P, NT], F32, tag="pst")
                    for k in range(src_kch):
                        nc.tensor.matmul(
                            ps[:slp_rows, :t1 - t0],
                            lhsT=w_cols_of_k(k)[:, :slp_rows],
                            rhs=src_getter(k, t0, t1),
                            start=(k == 0), stop=(k == src_kch - 1))
                    nc.scalar.copy(slp_dst[:slp_rows, t0:t1],
                                   ps[:slp_rows, :t1 - t0])
                for blk in range(nblk):
                    nb0 = blk * P
                    nb1 = min(nb0 + P, nloc)
                    nn = nb1 - nb0
                    ps = psp.tile([P, wcols], F32, tag="psnm")
                    for k in range(src_kch):
                        nc.tensor.matmul(
                            ps[:nn, :], lhsT=src_getter(k, nb0, nb1),
                            rhs=w_cols_of_k(k),
                            start=(k == 0), stop=(k == src_kch - 1))
                    tt = wp.tile([P, wcols], BF16, tag="tblt")
                    for (c0, c1, dcol) in prescale:
                        nc.vector.tensor_scalar_mul(
                            tt[:nn, c0:c1], ps[:nn, c0:c1],
                            dinv[:nn, dcol * nblk + blk: dcol * nblk + blk + 1])
                    nc.sync.dma_start(out=tbl_dst[nb0:nb1, 0:wcols], in_=tt[:nn, :])

            # ---------------------------------------------------------------
            def aggregate(tbl, tbl_cols, last, tail=None):
                """Gather + one-hot-matmul segment sums.
                layers 0/1 (last=False): raw sums into aggT (st rows 0:h,
                ts rows h:2h).  last=True: both branches into outTL[:c_out].
                tail(blocks) is emitted right after each supergroup's
                copies so post-processing / next-layer table building
                overlaps the next supergroup's gathers."""
                for sgi, blocks in enumerate(lay.sg_blocks):
                    bufs = {}
                    for b in range(2):
                        for hf in range(2):
                            ch0 = int(lay.half_chunk_off[b, blocks[0], hf])
                            ch1 = int(lay.half_chunk_off[b, blocks[-1], hf]
                                      + lay.cap_chunks[b, blocks[-1], hf])
                            nch = ch1 - ch0
                            if nch == 0:
                                continue
                            it = wp.tile([P, nch * 8], I16, tag=f"idx{b}{hf}")
                            nc.sync.dma_start(
                                out=it[:],
                                in_=idx_d[b, hf][:, ch0 * 8: ch1 * 8])
                            buf = mp.tile([P, nch, tbl_cols], BF16,
                                          tag=f"msg{b}{hf}")
                            nidx = nch * P
                            nc.gpsimd.dma_gather(
                                out_ap=buf[:], in_ap=tbl[hf * HALF:, :],
                                idxs_ap=it[:], num_idxs=nidx,
                                num_idxs_reg=nidx, elem_size=tbl_cols,
                                single_packet=(nidx <= 1024),
                                queue_num=next_q())
                            bufs[b, hf] = (buf, ch0)
                    # ---- masks: one broadcast DVE op per (branch, sg) ----
                    co = {}
                    malls = {}
                    for b in range(2):
                        c0 = int(lay.chunk_off[b, blocks[0], 0])
                        c1 = int(lay.chunk_off[b, blocks[-1], 1]
                                 + lay.cap_chunks[b, blocks[-1], 1])
                        co[b] = c0
                        nch_b = c1 - c0
                        if nch_b == 0:
                            continue
                        dt = wp.tile([P, nch_b, 1], BF16, tag=f"dt{b}")
                        nc.sync.dma_start(out=dt[:, :, 0],
                                          in_=dstloc_d[b][:, c0:c1])
                        mall = kp.tile([P, nch_b, P], BF16, tag="mask")
                        nc.vector.tensor_tensor(
                            out=mall[:],
                            in0=dt[:, :, 0:1].to_broadcast([P, nch_b, P]),
                            in1=iota3[:, 0:1, :].to_broadcast([P, nch_b, P]),
                            op=mybir.AluOpType.is_equal)
                        malls[b] = mall

                    def mask_col(b, cc):
                        return malls[b][:, cc - co[b], :]

                    for blk in blocks:
                        nb = slice(blk * P, min((blk + 1) * P, nblk_pad))
                        if last:
                            pss = {}
                        else:
                            ps2 = psp.tile([P, P], F32, tag="agg2")
                        wrote = [False, False]
                        for b in range(2):
                            chunks = []
                            for hf in range(2):
                                for j in range(int(lay.cap_chunks[b, blk, hf])):
                                    chunks.append((hf, j))
                            if not chunks:
                                continue
                            if last:
                                ps = psp.tile([c_out, P], F32,
                                              tag=("agg2" if b == 0 else "psnm"))
                                pss[b] = ps
                            for ci, (hf, j) in enumerate(chunks):
                                buf, ch0 = bufs[b, hf]
                                jj = (int(lay.half_chunk_off[b, blk, hf])
                                      - ch0 + j)
                                cc = int(lay.chunk_off[b, blk, hf]) + j
                                if last:
                                    lh = buf[:, jj, 0:c_out]
                                    o = ps[:, :]
                                    tpos = None
                                else:
                                    lh = buf[:, jj, b * h:(b + 1) * h]
                                    o = ps2[b * h:(b + 1) * h, :]
                                    tpos = (0, b * h)
                                nc.tensor.matmul(
                                    o, lhsT=lh, rhs=mask_col(b, cc),
                                    start=(ci == 0),
                                    stop=(ci == len(chunks) - 1),
                                    tile_position=tpos)
                            wrote[b] = True
                        if last:
                            if wrote[0]:
                                nc.scalar.copy(outTL[:, nb], pss[0][:, :])
                            else:
                                nc.vector.memset(outTL[:, nb], 0.0)
                            if wrote[1]:
                                nc.vector.tensor_add(out=outTL[:, nb],
                                                     in0=outTL[:, nb],
                                                     in1=pss[1][:, :])
                        else:
                            for b in range(2):
                                r = slice(b * h, (b + 1) * h)
                                if wrote[b]:
                                    nc.scalar.copy(aggT[r, nb], ps2[r, :])
                                else:
                                    nc.vector.memset(aggT[r, nb], 0.0)
                    if tail is not None:
                        tail(blocks)

            # ---------------------------------------------------------------
            def post01_slice(layer, out_tile, s0, s1):
                """out_tile = relu((aggT + xwT*dinvb) * dinvb + bias)"""
                nc.vector.tensor_tensor(out=xwT[:, s0:s1], in0=xwT[:, s0:s1],
                                        in1=dinvb[:, s0:s1],
                                        op=mybir.AluOpType.mult)
                nc.vector.tensor_tensor(out=aggT[:, s0:s1], in0=aggT[:, s0:s1],
                                        in1=xwT[:, s0:s1],
                                        op=mybir.AluOpType.add)
                nc.vector.tensor_tensor(out=aggT[:, s0:s1], in0=aggT[:, s0:s1],
                                        in1=dinvb[:, s0:s1],
                                        op=mybir.AluOpType.mult)
                nc.scalar.activation(out_tile[:, s0:s1], aggT[:, s0:s1],
                                     mybir.ActivationFunctionType.Relu,
                                     bias=biasv[:, layer:layer + 1])

            def tables_slice(hsrc, w_sb, wcols, slp_rows, slp_dst, tbl_dst,
                             prescale, blocks):
                """Next-layer table build restricted to a supergroup's
                node columns (src has kch=1)."""
                t0 = blocks[0] * P
                t1 = min(blocks[-1] * P + P, nloc)
                if t1 > t0:
                    ps = psp.tile([P, 512], F32, tag="pst")
                    nc.tensor.matmul(ps[:slp_rows, :t1 - t0],
                                     lhsT=w_sb[:, :slp_rows],
                                     rhs=hsrc[:, t0:t1],
                                     start=True, stop=True)
                    nc.scalar.copy(slp_dst[:slp_rows, t0:t1],
                                   ps[:slp_rows, :t1 - t0])
                for blk in blocks:
                    nb0 = blk * P
                    nb1 = min(nb0 + P, nloc)
                    if nb1 <= nb0:
                        continue
                    nn = nb1 - nb0
                    ps2 = psp.tile([P, wcols], F32, tag="psnm")
                    nc.tensor.matmul(ps2[:nn, :], lhsT=hsrc[:, nb0:nb1],
                                     rhs=w_sb[:, :wcols],
                                     start=True, stop=True)
                    tt = wp.tile([P, wcols], BF16, tag="tblt")
                    for (c0, c1, dcol) in prescale:
                        nc.vector.tensor_scalar_mul(
                            tt[:nn, c0:c1], ps2[:nn, c0:c1],
                            dinv[:nn, dcol * nblk + blk: dcol * nblk + blk + 1])
                    nc.sync.dma_start(out=tbl_dst[nb0:nb1, 0:wcols],
                                      in_=tt[:nn, :])

            def final_slice(blocks):
                """Last-layer epilogue for a supergroup: normalization,
                bias, then per-block log_softmax and output DMA."""
                s0 = blocks[0] * P
                s1 = min(blocks[-1] * P + P, nblk_pad)
                nc.vector.tensor_tensor(out=xwTL[:, s0:s1], in0=xwTL[:, s0:s1],
                                        in1=dinvallb[:, s0:s1],
                                        op=mybir.AluOpType.mult)
                nc.vector.tensor_tensor(out=outTL[:, s0:s1], in0=outTL[:, s0:s1],
                                        in1=xwTL[:, s0:s1],
                                        op=mybir.AluOpType.add)
                nc.vector.tensor_tensor(out=outTL[:, s0:s1], in0=outTL[:, s0:s1],
                                        in1=dinvallb[:, s0:s1],
                                        op=mybir.AluOpType.mult)
                nc.scalar.activation(outTL[:, s0:s1], outTL[:, s0:s1],
                                     mybir.ActivationFunctionType.Identity,
                                     bias=biasL[:, 0:1])
                for blk in blocks:
                    nb0 = blk * P
                    nb1 = min(nb0 + P, nloc)
                    if nb1 <= nb0:
                        continue
                    nn = nb1 - nb0
                    tp = psp.tile([P, c_out], F32, tag="psnm")
                    nc.tensor.transpose(tp[:], outTL[:, nb0:nb0 + P],
                                        ident[:c_out, :c_out])
                    negmax = wp.tile([P, 1], F32, tag="negmax")
                    nc.vector.tensor_reduce(negmax[:], tp[:],
                                            axis=mybir.AxisListType.X,
                                            op=mybir.AluOpType.max, negate=True)
                    ex = wp.tile([P, c_out], F32, tag="ex")
                    nc.scalar.activation(ex[:], tp[:],
                                         mybir.ActivationFunctionType.Exp,
                                         bias=negmax[:, 0:1])
                    sume = wp.tile([P, 1], F32, tag="sume")
                    nc.vector.tensor_reduce(sume[:], ex[:],
                                            axis=mybir.AxisListType.X,
                                            op=mybir.AluOpType.add)
                    lse = wp.tile([P, 1], F32, tag="lse")
                    nc.scalar.activation(lse[:], sume[:],
                                         mybir.ActivationFunctionType.Ln)
                    fin = wp.tile([P, c_out], F32, tag="fin")
                    nc.vector.tensor_scalar(
                        out=fin[:], in0=tp[:], scalar1=negmax[:, 0:1],
                        scalar2=lse[:, 0:1], op0=mybir.AluOpType.add,
                        op1=mybir.AluOpType.subtract)
                    nc.sync.dma_start(out=out_d[nb0:nb1, :], in_=fin[:nn, :])

            def early_out(tile_ap):
                # debug escape hatch: dump a [128,c_out] sample and stop
                nc.sync.dma_start(out=out_d[0:P, :], in_=tile_ap)

            def _phases():
                if stop_at <= 1:
                    early_out(dinvb[0:P, 0:c_out])
                if nblk_pad > nloc:
                    nc.vector.memset(xwT[:, nloc:], 0.0)
                    nc.vector.memset(hT[:, nloc:], 0.0)
                    nc.vector.memset(h2T[:, nloc:], 0.0)
                # =================== layer 0 ===================
                def x_loader(k, j0, j1):
                    t = wp.tile([P, 512], BF16, tag="xk")
                    nc.sync.dma_start(
                        out=t[:, :j1 - j0],
                        in_=xT_d[:, k * nloc + j0: k * nloc + j1])
                    return t[:, :j1 - j0]

                if stop_at <= 1:
                    return
                with nc.named_scope("l0_tables"):
                    build_tables(
                        src_getter=x_loader,
                        src_kch=kch,
                        w_cols_of_k=lambda k: w0_sb[:, k * h2:(k + 1) * h2],
                        wcols=h2, slp_rows=h2, slp_dst=xwT, tbl_dst=tbl_loc[0],
                        prescale=[(0, h, 0), (h, h2, 1)])
                if stop_at <= 2:
                    early_out(xwT[0:P, 0:c_out])
                    return
                with nc.named_scope("l0_allgather"):
                    barrier()
                    nc.gpsimd.collective_compute(
                        "AllGather", mybir.AluOpType.bypass,
                        replica_groups=[core_ids],
                        ins=[tbl_loc[0][:]], outs=[tbl_full[0][:]])
                    barrier()
                if stop_at <= 3:
                    gdbg = wp.tile([P, c_out], BF16, tag="gdbg")
                    nc.sync.dma_start(out=gdbg[:], in_=tbl_full[0][0:P, 0:c_out])
                    gdbgf = wp.tile([P, c_out], F32, tag="gdbgf")
                    nc.vector.tensor_copy(gdbgf[:], gdbg[:])
                    early_out(gdbgf[:])
                    return

                # agg(l0) with interleaved post + layer-1 tables per sg
                def tail0(blocks):
                    s0 = blocks[0] * P
                    s1 = min(blocks[-1] * P + P, nblk_pad)
                    post01_slice(0, hT, s0, s1)
                    tables_slice(hT, w1_sb, h2, h2, xwT, tbl_loc[1],
                                 [(0, h, 0), (h, h2, 1)], blocks)

                with nc.named_scope("l0_agg"):
                    aggregate(tbl_full[0], h2, last=False, tail=tail0)
                if stop_at <= 5:
                    hdbg = wp.tile([P, c_out], F32, tag="hdbg")
                    nc.vector.tensor_copy(hdbg[:], hT[0:P, 0:c_out])
                    early_out(hdbg[:])
                    return

                # =================== layer 1 ===================
                with nc.named_scope("l1_allgather"):
                    barrier()
                    nc.gpsimd.collective_compute(
                        "AllGather", mybir.AluOpType.bypass,
                        replica_groups=[core_ids],
                        ins=[tbl_loc[1][:]], outs=[tbl_full[1][:]])
                    barrier()

                def tail1(blocks):
                    s0 = blocks[0] * P
                    s1 = min(blocks[-1] * P + P, nblk_pad)
                    post01_slice(1, h2T, s0, s1)
                    tables_slice(h2T, wl_sb, 128, c_out, xwTL, tblL_loc,
                                 [(0, 128, 2)], blocks)

                with nc.named_scope("l1_agg"):
                    aggregate(tbl_full[1], h2, last=False, tail=tail1)
                if stop_at <= 8:
                    hdbg2 = wp.tile([P, c_out], F32, tag="hdbg")
                    nc.vector.tensor_copy(hdbg2[:], h2T[0:P, 0:c_out])
                    early_out(hdbg2[:])
                    return

                # =================== last layer ===================
                with nc.named_scope("l2_allgather"):
                    barrier()
                    nc.gpsimd.collective_compute(
                        "AllGather", mybir.AluOpType.bypass,
                        replica_groups=[core_ids],
                        ins=[tblL_loc[:]], outs=[tblL_full[:]])
                    barrier()
                with nc.named_scope("l2_agg"):
                    aggregate(tblL_full, 128, last=True, tail=final_slice)

            _phases()

    nc.compile()
    return nc


# ----------------------------------------------------------------------------
# driver
# ----------------------------------------------------------------------------

_CACHE = {}
_RUNNER = {}


def _build_runner(nc, n_cores):
    """Persistent jitted executor (no donation; inputs stay on device)."""
    import jax
    from jax.sharding import Mesh, PartitionSpec
    from jax.experimental.shard_map import shard_map
    import concourse.mybir as mybir_
    from concourse import bass2jax
    from concourse.bass2jax import _bass_exec_p, partition_id_tensor

    bass2jax.install_neuronx_cc_hook()
    partition_name = (nc.partition_id_tensor.name
                      if nc.partition_id_tensor else None)
    in_names, out_names, out_avals, zero_outs = [], [], [], []
    for alloc in nc.m.functions[0].allocations:
        if not isinstance(alloc, mybir_.MemoryLocationSet):
            continue
        name = alloc.memorylocations[0].name
        if alloc.kind == "ExternalInput":
            if name != partition_name:
                in_names.append(name)
        elif alloc.kind == "ExternalOutput":
            out_names.append(name)
            shape = tuple(alloc.tensor_shape)
            dtype = mybir_.dt.np(alloc.dtype)
            out_avals.append(jax.core.ShapedArray(shape, dtype))
            zero_outs.append(np.zeros(shape, dtype))
    n_params = len(in_names)
    all_names = in_names + out_names
    if partition_name is not None:
        all_names.append(partition_name)

    def _body(*args):
        operands = list(args)
        if partition_name is not None:
            operands.append(partition_id_tensor())
        return tuple(_bass_exec_p.bind(
            *operands, out_avals=tuple(out_avals), in_names=tuple(all_names),
            out_names=tuple(out_names), lowering_input_output_aliases=(),
            sim_require_finite=True, sim_require_nnan=True, nc=nc))

    devices = jax.devices()[:n_cores]
    mesh = Mesh(np.asarray(devices), ("core",))
    n_out = len(out_names)
    fn = jax.jit(shard_map(_body, mesh=mesh,
                           in_specs=(PartitionSpec("core"),) * (n_params + n_out),
                           out_specs=(PartitionSpec("core"),) * n_out,
                           check_rep=False), keep_unused=True)
    return fn, in_names, out_names, out_avals, zero_outs, mesh


def _run_persistent(nc, in_maps, n_cores, key):
    import jax
    if key not in _RUNNER:
        fn, in_names, out_names, out_avals, zero_outs, mesh = \
            _build_runner(nc, n_cores)
        _RUNNER[key] = dict(fn=fn, in_names=in_names, out_names=out_names,
                            out_avals=out_avals, zero_outs=zero_outs,
                            mesh=mesh, dev_args=None)
    R = _RUNNER[key]
    concat_in = [np.concatenate([np.asarray(in_maps[c][nm])
                                 for c in range(n_cores)], axis=0)
                 for nm in R["in_names"]]
    concat_zero = [np.zeros((n_cores * z.shape[0], *z.shape[1:]), z.dtype)
                   for z in R["zero_outs"]]
    args = [jax.device_put(a) for a in concat_in + concat_zero]
    R["dev_args"] = args
    outs = R["fn"](*args)
    outs = [np.asarray(o) for o in outs]
    return {nm: outs[i].reshape(n_cores, *R["out_avals"][i].shape)
            for i, nm in enumerate(R["out_names"])}


# ---------------------------------------------------------------------------
# device timing: NTFF (neuron-profile) with difference-method fallback
# ---------------------------------------------------------------------------

def _axon_profile_hook(so_path="/opt/axon/libaxon_pjrt.so"):
    try:
        lib = ctypes.CDLL(so_path)
    except OSError:
        return None
    if not hasattr(lib, "axon_start_nrt_profile"):
        return None
    lib.axon_start_nrt_profile.argtypes = [ctypes.POINTER(ctypes.c_int64),
                                           ctypes.c_size_t]
    lib.axon_start_nrt_profile.restype = ctypes.c_int64
    lib.axon_stop_nrt_profile.argtypes = [ctypes.c_char_p]
    lib.axon_stop_nrt_profile.restype = ctypes.c_int64

    @contextlib.contextmanager
    def _hook(output_dir, device_ids):
        import jax
        jax.devices()
        if device_ids:
            ids = (ctypes.c_int64 * len(device_ids))(*device_ids)
            rc = lib.axon_start_nrt_profile(ids, len(device_ids))
        else:
            rc = lib.axon_start_nrt_profile(None, 0)
        if rc != 0:
            raise RuntimeError(f"axon_start_nrt_profile rc={rc}")
        try:
            yield
        finally:
            n = lib.axon_stop_nrt_profile(str(output_dir).encode())
            if n <= 0:
                print(f"profile capture wrote {n} files", file=sys.stderr)

    return _hook


def _ntff_exec_ns(key, devices=(0,)):
    """Profile one warm execution; return max NEFF device span in ns."""
    import jax
    hook = _axon_profile_hook()
    if hook is None:
        return None
    R = _RUNNER[key]
    fn, args = R["fn"], R["dev_args"]
    o = fn(*args)
    jax.block_until_ready(o)
    tmpdir = tempfile.mkdtemp(prefix="ntff_timing_")
    with hook(tmpdir, list(devices)):
        o = fn(*args)
        jax.block_until_ready(o)
    ntffs = sorted(glob.glob(os.path.join(tmpdir, "*_body*device*.ntff")))
    if not ntffs:
        return None
    times = []
    for i, ntff in enumerate(ntffs):
        # pair the ntff with its own executable's neff by name prefix
        prefix = os.path.basename(ntff).split("-device")[0]
        neffs = glob.glob(os.path.join(tmpdir, prefix + ".neff"))
        if not neffs:
            continue
        outj = os.path.join(tmpdir, f"prof_{i}.json")
        try:
            subprocess.run(
                ["neuron-profile", "view", "-n", neffs[0], "-s", ntff,
                 "--output-format=json", "--output-file", outj,
                 "--ignore-nc-buf-usage"],
                check=True, capture_output=True,
                env=dict(os.environ, NEURON_PROFILE_DBG_OUTPUT="2"))
            with open(outj) as f:
                j = json.load(f)
            times.append(float(j["summary"][0]["total_time"]) * 1e9)
        except Exception as e:
            print("neuron-profile failed:", e, file=sys.stderr)
    return max(times) if times else None


def time_device(inputs, iters=6, cfg=None):
    """Device execution time in ns.

    Primary: neuron-profile (NTFF) span of one warm on-device execution.
    Fallback: marginal per-execution wall time via the difference method."""
    import jax, time as _t
    cfg = cfg or FULL_CFG
    key = _ensure_built(cfg, inputs)
    if os.environ.get("BASS_TIME_MODE", "ntff") == "ntff":
        try:
            ns = _ntff_exec_ns(key)
        except Exception as e:
            print("ntff timing failed:", e, file=sys.stderr)
            ns = None
        if ns is not None:
            return ns
    R = _RUNNER[key]
    fn, args = R["fn"], R["dev_args"]
    o = fn(*args); jax.block_until_ready(o)
    t0 = _t.time()
    o = fn(*args); jax.block_until_ready(o)
    t1 = _t.time()
    base = t1 - t0
    t0 = _t.time()
    for _ in range(1 + iters):
        o = fn(*args)
    jax.block_until_ready(o)
    t1 = _t.time()
    per = (t1 - t0 - base) / iters
    return per * 1e9


def _ensure_built(cfg, inputs):
    weights = {k: np.asarray(inputs[k]) for k in
               ("W_st0", "b_st0", "W_ts0", "b_ts0", "W_st1", "b_st1",
                "W_ts1", "b_ts1", "W_last", "b_last")}
    lay, in_maps = host_prep(cfg, inputs["x"], inputs["edge_index"],
                             inputs["is_reversed"])
    wmap = host_prep_weights(cfg, **weights)
    for m in in_maps:
        m.update(wmap)
    sig = (lay.signature(), 99, "full")
    if sig not in _CACHE:
        _CACHE[sig] = build_program(cfg, lay, stop_at=99)
    nc = _CACHE[sig]
    key = str(sig)
    _run_persistent(nc, in_maps, cfg["n_cores"], key)
    return key


def run(cfg, x, edge_index, is_reversed, weights, use_sim=False, stop_at=99,
        agg_mode="full"):
    lay, in_maps = host_prep(cfg, x, edge_index, is_reversed)
    wmap = host_prep_weights(cfg, **weights)
    for m in in_maps:
        m.update(wmap)

    sig = (lay.signature(), stop_at, agg_mode)
    if sig in _CACHE:
        nc = _CACHE[sig]
    else:
        nc = build_program(cfg, lay, stop_at=stop_at)
        _CACHE[sig] = nc

    n_cores = cfg["n_cores"]
    if use_sim:
        import concourse.bass_interp as bass_interp
        sim = bass_interp.MultiCoreSim(nc, n_cores, require_finite=False,
                                       require_nnan=False)
        for c in range(n_cores):
            for k, v in in_maps[c].items():
                sim.cores[c].tensor(k)[:] = v
        sim.simulate()
        outs = [np.array(sim.cores[c].tensor("out")) for c in range(n_cores)]
    else:
        key = str(sig)
        res = _run_persistent(nc, in_maps, n_cores, key)
        outs = list(res["out"])
    return np.concatenate(outs, axis=0)


def kernel(x, edge_index, is_reversed, W_st0, b_st0, W_ts0, b_ts0,
           W_st1, b_st1, W_ts1, b_ts1, W_last, b_last):
    cfg = FULL_CFG
    weights = dict(W_st0=W_st0, b_st0=b_st0, W_ts0=W_ts0, b_ts0=b_ts0,
                   W_st1=W_st1, b_st1=b_st1, W_ts1=W_ts1, b_ts1=b_ts1,
                   W_last=W_last, b_last=b_last)
    out = run(cfg, x, edge_index, is_reversed, weights)
    return out.astype(np.float32)


# revision 23
# speedup vs baseline: 1.1726x; 1.1726x over previous
"""Trainium2 Bass kernel for nn_BiModel (2-layer bidirectional GCN).

Distribution over 8 NeuronCores: nodes sharded 6250/core, edge lists
partitioned by destination core and sorted by (branch, dst-block,
src-half).  Per GCN layer each core computes the transformed features
for its own nodes (prescaled by dinv[src]), AllGathers the bf16
message table, dma_gathers the src rows of its edges and segment-sums
them with one-hot matmuls on the tensor engine (PSUM accumulation per
128-node destination block).  Host work is limited to sharding/layout
prep (transpose/pad, edge partition+sort, degree counts, gather index
tensors); all floating-point math on tensor data runs on device.

v2 perf changes vs baseline:
- exact data-driven chunk capacities (no floors): ~16% fewer gather
  descriptors / masks / matmuls
- dma_gather descriptor generation round-robins 4 SWDGE queues (2-way
  parallel Q7 generation measured on HW)
- one-hot masks built in ONE broadcast-AP tensor_tensor per
  (branch-pair, supergroup) instead of one DVE op per chunk
- gathers read the AllGather output directly (no DRAM->DRAM copy)
- message buffers double-buffered across supergroups
"""

import contextlib
import ctypes
import glob
import json
import os
import subprocess
import sys
import tempfile
import types

import numpy as np

import concourse.bass as bass
import concourse.bacc as bacc
import concourse.mybir as mybir
import concourse.tile as tile
from concourse.bass_utils import run_bass_kernel_spmd
from concourse.masks import make_identity

import ml_dtypes

P = 128
F32 = mybir.dt.float32
BF16 = mybir.dt.bfloat16
I16 = mybir.dt.int16
I32 = mybir.dt.int32
HALF = 32768

FULL_CFG = dict(n=50000, e=800000, f_in=500, h=64, c_out=16, n_cores=8,
                sb_blocks=4, cap_floor=None, nqueues=4)


def cdiv(a, b):
    return (a + b - 1) // b


# ----------------------------------------------------------------------------
# host-side layout / preprocessing
# ----------------------------------------------------------------------------

class Layout:
    """Compile-time chunk layout, shared by all cores (uniform SPMD
    program).  Group = (branch, dst-block, src-half); per-group chunk
    capacity = max edge count over cores, rounded up to 128."""

    def __init__(self, cfg, group_counts):
        # group_counts: [n_cores, 2, nblk, 2]
        self.cfg = cfg
        self.nloc = cfg["n"] // cfg["n_cores"]
        self.nblk = cdiv(self.nloc, P)
        cap = group_counts.max(axis=0)                    # [2, nblk, 2]
        self.cap_chunks = cdiv(cap, P)                    # may be 0
        floor = cfg.get("cap_floor")
        if floor is not None:
            flo, fhi = floor
            self.cap_chunks = np.maximum(
                self.cap_chunks,
                np.array([[[flo, fhi]]], np.int64))
        # branch-major chunk stream (for dstloc): (blk, half) order
        self.chunk_off = np.zeros((2, self.nblk, 2), np.int64)
        for b in range(2):
            off = 0
            for blk in range(self.nblk):
                for hf in range(2):
                    self.chunk_off[b, blk, hf] = off
                    off += self.cap_chunks[b, blk, hf]
        self.nchunks_br = self.chunk_off[:, -1, 1] + self.cap_chunks[:, -1, 1]
        # per-(branch, half) gather stream: blk order
        self.half_chunk_off = np.zeros((2, self.nblk, 2), np.int64)
        self.nchunks_bh = np.zeros((2, 2), np.int64)
        for b in range(2):
            for hf in range(2):
                off = 0
                for blk in range(self.nblk):
                    self.half_chunk_off[b, blk, hf] = off
                    off += self.cap_chunks[b, blk, hf]
                self.nchunks_bh[b, hf] = off
        sb = cfg["sb_blocks"]
        self.sg_blocks = [list(range(g * sb, min((g + 1) * sb, self.nblk)))
                          for g in range(cdiv(self.nblk, sb))]

    def signature(self):
        return (tuple(self.cap_chunks.reshape(-1).tolist()),
                tuple(sorted((k, str(v)) for k, v in self.cfg.items())))


def _wrap_idx16(idx, n_pad):
    buf = np.zeros(n_pad, np.int16)
    buf[: len(idx)] = idx.astype(np.int16)
    w = buf.reshape(n_pad // 16, 16).T            # [16, n/16]
    return np.ascontiguousarray(np.tile(w, (8, 1)))  # [128, n/16]


def host_prep(cfg, x, edge_index, is_reversed):
    n, f_in = cfg["n"], cfg["f_in"]
    n_cores = cfg["n_cores"]
    nloc = n // n_cores
    f_pad = cdiv(f_in, P) * P

    src = np.asarray(edge_index[0], np.int64)
    dst = np.asarray(edge_index[1], np.int64)
    rev = np.asarray(is_reversed).astype(bool)

    core = dst // nloc
    dl = dst % nloc
    blk = dl // P
    sblk = (dl % P).astype(np.float32)
    branch = rev.astype(np.int64)
    hf = (src >= HALF).astype(np.int64)

    nblk = cdiv(nloc, P)
    key = ((core * 2 + branch) * nblk + blk) * 2 + hf
    order = np.argsort(key, kind="stable")
    counts = np.bincount(key[order], minlength=n_cores * 2 * nblk * 2)
    counts = counts.reshape(n_cores, 2, nblk, 2)
    lay = Layout(cfg, counts)

    deg = np.zeros((2, n), np.float32)
    np.add.at(deg[0], dst[~rev], 1.0)
    np.add.at(deg[1], dst[rev], 1.0)

    xT = np.zeros((f_pad, n), ml_dtypes.bfloat16)
    xT[:f_in] = np.asarray(x, np.float32).T
    kch = f_pad // P

    src_s = src[order]
    sblk_s = sblk[order]
    gs = np.concatenate([[0], np.cumsum(counts.reshape(-1))])[:-1]
    gs = gs.reshape(n_cores, 2, nblk, 2)

    nblk_pad = nblk * P
    in_maps = []
    for c in range(n_cores):
        xc = xT[:, c * nloc:(c + 1) * nloc].reshape(kch, P, nloc)
        m = {"xT": np.ascontiguousarray(
            xc.transpose(1, 0, 2).reshape(P, kch * nloc))}
        degs = np.ones((P, 2 * nblk), np.float32)
        for b in range(2):
            dloc = np.ones(nblk_pad, np.float32)
            dloc[:nloc] = deg[b, c * nloc:(c + 1) * nloc]
            degs[:, b * nblk:(b + 1) * nblk] = dloc.reshape(nblk, P).T
        m["degs"] = degs
        for b in range(2):
            ncol = max(int(lay.nchunks_br[b]), 1)
            dst_cols = np.full((P, ncol), -1.0, np.float32)
            for hf_ in range(2):
                tot = max(int(lay.nchunks_bh[b, hf_]), 1) * P
                idx_stream = np.zeros(tot, np.int16)
                for blk_ in range(lay.nblk):
                    cnt = int(counts[c, b, blk_, hf_])
                    s0 = int(gs[c, b, blk_, hf_])
                    ho = int(lay.half_chunk_off[b, blk_, hf_]) * P
                    idx_stream[ho:ho + cnt] = src_s[s0:s0 + cnt] - hf_ * HALF
                    co = int(lay.chunk_off[b, blk_, hf_])
                    ce = int(lay.cap_chunks[b, blk_, hf_])
                    if ce:
                        dv = np.full(ce * P, -1.0, np.float32)
                        dv[:cnt] = sblk_s[s0:s0 + cnt]
                        dst_cols[:, co:co + ce] = dv.reshape(ce, P).T
                m[f"idx_b{b}h{hf_}"] = _wrap_idx16(idx_stream, tot)
            m[f"dstloc_b{b}"] = dst_cols.astype(ml_dtypes.bfloat16)
        in_maps.append(m)
    return lay, in_maps


def host_prep_weights(cfg, W_st0, b_st0, W_ts0, b_ts0, W_st1, b_st1,
                      W_ts1, b_ts1, W_last, b_last):
    f_in, h, c_out = cfg["f_in"], cfg["h"], cfg["c_out"]
    f_pad = cdiv(f_in, P) * P
    W0 = np.zeros((f_pad, 2 * h), np.float32)
    W0[:f_in, :h] = W_st0
    W0[:f_in, h:] = W_ts0
    kch = f_pad // P
    W0 = np.ascontiguousarray(
        W0.reshape(kch, P, 2 * h).transpose(1, 0, 2).reshape(P, kch * 2 * h))
    W1 = np.concatenate([W_st1, W_ts1], axis=1).astype(np.float32)
    WL = np.zeros((2 * h, 128), np.float32)
    WL[:, :c_out] = W_last
    bias01 = np.stack([np.concatenate([b_st0, b_ts0]),
                       np.concatenate([b_st1, b_ts1])], axis=1).astype(np.float32)
    return dict(W0=W0.astype(ml_dtypes.bfloat16),
                W1=W1.astype(ml_dtypes.bfloat16),
                WL=WL.astype(ml_dtypes.bfloat16), bias01=bias01,
                b_last=np.asarray(b_last, np.float32).reshape(c_out, 1))


# ----------------------------------------------------------------------------
# device program
# ----------------------------------------------------------------------------

def build_program(cfg, lay, stop_at=99, agg_mode="full"):
    n, f_in = cfg["n"], cfg["f_in"]
    h, c_out = cfg["h"], cfg["c_out"]
    n_cores = cfg["n_cores"]
    nqueues = cfg.get("nqueues", 4)
    nloc = n // n_cores
    nblk = lay.nblk
    nblk_pad = nblk * P
    f_pad = cdiv(f_in, P) * P
    kch = f_pad // P
    h2 = 2 * h
    core_ids = list(range(n_cores))

    nc = bacc.Bacc("TRN2", target_bir_lowering=False, debug=False,
                   num_devices=n_cores, num_swdge_queues=nqueues)

    xT_d = nc.declare_dram_parameter("xT", [P, kch * nloc], BF16, isOutput=False)
    degs_d = nc.declare_dram_parameter("degs", [P, 2 * nblk], F32, isOutput=False)
    W0_d = nc.declare_dram_parameter("W0", [P, kch * h2], BF16, isOutput=False)
    W1_d = nc.declare_dram_parameter("W1", [h2, h2], BF16, isOutput=False)
    WL_d = nc.declare_dram_parameter("WL", [h2, 128], BF16, isOutput=False)
    bias01_d = nc.declare_dram_parameter("bias01", [h2, 2], F32, isOutput=False)
    b_last_d = nc.declare_dram_parameter("b_last", [c_out, 1], F32, isOutput=False)
    idx_d = {}
    for b in range(2):
        for hf in range(2):
            w = max(int(lay.nchunks_bh[b, hf]), 1) * 8
            idx_d[b, hf] = nc.declare_dram_parameter(
                f"idx_b{b}h{hf}", [P, w], I16, isOutput=False)
    dstloc_d = [nc.declare_dram_parameter(
        f"dstloc_b{b}", [P, max(int(lay.nchunks_br[b]), 1)], BF16,
        isOutput=False) for b in range(2)]
    out_d = nc.declare_dram_parameter("out", [nloc, c_out], F32, isOutput=True)

    tbl_loc = [nc.dram_tensor(f"tbl_loc{i}", [nloc, h2], BF16) for i in range(2)]
    tbl_full = [nc.dram_tensor(f"tbl_full{i}", [n, h2], BF16,
                               addr_space="Shared") for i in range(2)]
    nblk_pad_ = cdiv(n // n_cores, P) * P
    dinv_flat_d = nc.dram_tensor("dinv_flat", [1, 3 * nblk_pad_], F32)
    tblL_loc = nc.dram_tensor("tblL_loc", [nloc, 128], BF16)
    tblL_full = nc.dram_tensor("tblL_full", [n, 128], BF16, addr_space="Shared")

    qctr = [0]

    def next_q():
        q = qctr[0] % nqueues
        qctr[0] += 1
        return q

    with tile.TileContext(nc) as tc:
        with (
            tc.tile_pool(name="persist", bufs=1) as pp,
            tc.tile_pool(name="work", bufs=2) as wp,
            tc.tile_pool(name="msg", bufs=3) as mp,
            tc.tile_pool(name="mask", bufs=2) as kp,
            tc.tile_pool(name="psum", bufs=2, space="PSUM") as psp,
            tc.tile_pool(name="psagg", bufs=3, space="PSUM") as psa,
        ):
            # ---------- constants ----------
            iota_i = wp.tile([P, P], I32, tag="ioi")
            nc.gpsimd.iota(iota_i[:], pattern=[[1, P]], base=0,
                           channel_multiplier=0)
            iota3 = pp.tile([P, 1, P], BF16, tag="io3")
            nc.vector.tensor_copy(iota3[:, 0, :], iota_i[:])
            ident = pp.tile([P, P], F32, tag="ident")
            make_identity(nc, ident[:])
            biasv = pp.tile([P, 2], F32, tag="biasv")
            nc.sync.dma_start(out=biasv[:], in_=bias01_d[:, :])
            biasL = pp.tile([c_out, 1], F32, tag="biasL")
            nc.sync.dma_start(out=biasL[:], in_=b_last_d[:, :])

            def barrier():
                tc.strict_bb_all_engine_barrier()

            # ---------- degrees -> dinv [128, 3*nblk] (st|ts|all) ----------
            deg_sb = wp.tile([P, 2 * nblk], F32, tag="degsb")
            nc.sync.dma_start(out=deg_sb[:], in_=degs_d[:, :])
            dtmp = wp.tile([P, 3 * nblk], F32, tag="dtmp")
            nc.vector.tensor_tensor(out=dtmp[:, 2 * nblk:],
                                    in0=deg_sb[:, :nblk], in1=deg_sb[:, nblk:],
                                    op=mybir.AluOpType.add)
            nc.vector.tensor_copy(dtmp[:, :2 * nblk], deg_sb[:])
            nc.vector.tensor_scalar_add(dtmp[:], dtmp[:], 1.0)
            dsq = wp.tile([P, 3 * nblk], F32, tag="dsq")
            nc.scalar.sqrt(dsq[:], dtmp[:])
            dinv = pp.tile([P, 3 * nblk], F32, tag="dinv")
            nc.vector.reciprocal(dinv[:], dsq[:])

            # transposed rows: dinvT [nblk, 3*128]
            dinvT = pp.tile([nblk, 3 * P], F32, tag="dinvT")
            for i in range(3):
                tps = psp.tile([nblk, P], F32, tag="pst")
                nc.tensor.transpose(tps[:], dinv[:, i * nblk:(i + 1) * nblk],
                                    ident[:])
                nc.scalar.copy(dinvT[:, i * P:(i + 1) * P], tps[:])

            # flatten dinvT rows into DRAM [1, 3*nblk_pad]
            for i in range(3):
                nc.sync.dma_start(
                    out=dinv_flat_d[0:1, i * nblk_pad:(i + 1) * nblk_pad],
                    in_=dinvT[:, i * P:(i + 1) * P])

            # broadcast tiles via K=1 matmul: ones[1,M].T @ row[1,N]
            ones_row = pp.tile([1, P], F32, tag="ones_row")
            nc.vector.memset(ones_row[:], 1.0)
            dinvb = pp.tile([P, nblk_pad], BF16, tag="dinvb")
            dinvallb = pp.tile([c_out, nblk_pad], BF16, tag="dinvallb")
            NTB = 512
            for t0 in range(0, nblk_pad, NTB):
                t1 = min(t0 + NTB, nblk_pad)
                dfs = wp.tile([1, 3 * NTB], F32, tag="dfs")
                for i in range(3):
                    nc.sync.dma_start(
                        out=dfs[0:1, i * NTB: i * NTB + t1 - t0],
                        in_=dinv_flat_d[0:1, i * nblk_pad + t0: i * nblk_pad + t1])
                bps = psp.tile([P, NTB], F32, tag="pst")
                nc.tensor.matmul(bps[0:h, :t1 - t0], lhsT=ones_row[0:1, 0:h],
                                 rhs=dfs[0:1, 0:t1 - t0],
                                 start=True, stop=True)
                nc.tensor.matmul(bps[h:h2, :t1 - t0], lhsT=ones_row[0:1, 0:h],
                                 rhs=dfs[0:1, NTB:NTB + t1 - t0],
                                 start=True, stop=True, tile_position=(0, h))
                nc.scalar.copy(dinvb[:, t0:t1], bps[:, :t1 - t0])
                bps2 = psp.tile([c_out, NTB], F32, tag="psnm")
                nc.tensor.matmul(bps2[:, :t1 - t0], lhsT=ones_row[0:1, 0:c_out],
                                 rhs=dfs[0:1, 2 * NTB:2 * NTB + t1 - t0],
                                 start=True, stop=True)
                nc.scalar.copy(dinvallb[:, t0:t1], bps2[:, :t1 - t0])

            # ---------- weights ----------
            w0_sb = pp.tile([P, kch * h2], BF16, tag="w0")
            nc.sync.dma_start(out=w0_sb[:], in_=W0_d[:, :])
            w1_sb = pp.tile([P, h2], BF16, tag="w1")
            nc.sync.dma_start(out=w1_sb[:], in_=W1_d[:, :])
            wl_sb = pp.tile([P, 128], BF16, tag="wl")
            nc.sync.dma_start(out=wl_sb[:], in_=WL_d[:, :])

            # ---------- edge metadata, loaded once (shared by all layers) --
            idx_sb = {}
            for b in range(2):
                for hf in range(2):
                    w = max(int(lay.nchunks_bh[b, hf]), 1) * 8
                    t = pp.tile([P, w], I16, tag=f"idxpre{b}{hf}")
                    nc.sync.dma_start(out=t[:], in_=idx_d[b, hf][:, :])
                    idx_sb[b, hf] = t
            dt_sb = []
            for b in range(2):
                ncol = max(int(lay.nchunks_br[b]), 1)
                t = pp.tile([P, ncol, 1], BF16, tag=f"dtpre{b}")
                nc.sync.dma_start(out=t[:, :, 0], in_=dstloc_d[b][:, :])
                dt_sb.append(t)

            # ---------- state ----------
            hT = pp.tile([P, nblk_pad], BF16, tag="hT")
            h2T = hT  # layer-1 output reuses the buffer (dead after use)
            xwT = pp.tile([P, nblk_pad], BF16, tag="xwT")
            aggT = pp.tile([P, nblk_pad], F32, tag="aggT")
            # last-layer [16,*] views over tiles that are dead by then
            xwTL = xwT[0:c_out, :]
            outTL = aggT[0:c_out, :]

            # ---------------------------------------------------------------
            def build_tables(src_getter, src_kch, w_cols_of_k, wcols,
                             slp_rows, slp_dst, tbl_dst, prescale):
                """src_getter(k, j0, j1) -> [128, j1-j0] AP of input chunk k;
                w_cols_of_k(k) -> [128, wcols] weight AP.
                Writes transposed xw to slp_dst[:slp_rows] and prescaled
                bf16 node-major rows to tbl_dst."""
                NT = 512
                for t0 in range(0, nloc, NT):
                    t1 = min(t0 + NT, nloc)
                    ps = psp.tile([P, NT], F32, tag="pst")
                    for k in range(src_kch):
                        nc.tensor.matmul(
                            ps[:slp_rows, :t1 - t0],
                            lhsT=w_cols_of_k(k)[:, :slp_rows],
                            rhs=src_getter(k, t0, t1),
                            start=(k == 0), stop=(k == src_kch - 1))
                    nc.scalar.copy(slp_dst[:slp_rows, t0:t1],
                                   ps[:slp_rows, :t1 - t0])
                for blk in range(nblk):
                    nb0 = blk * P
                    nb1 = min(nb0 + P, nloc)
                    nn = nb1 - nb0
                    ps = psp.tile([P, wcols], F32, tag="psnm")
                    for k in range(src_kch):
                        nc.tensor.matmul(
                            ps[:nn, :], lhsT=src_getter(k, nb0, nb1),
                            rhs=w_cols_of_k(k),
                            start=(k == 0), stop=(k == src_kch - 1))
                    tt = wp.tile([P, wcols], BF16, tag="tblt")
                    for (c0, c1, dcol) in prescale:
                        nc.vector.tensor_scalar_mul(
                            tt[:nn, c0:c1], ps[:nn, c0:c1],
                            dinv[:nn, dcol * nblk + blk: dcol * nblk + blk + 1])
                    nc.sync.dma_start(out=tbl_dst[nb0:nb1, 0:wcols], in_=tt[:nn, :])

            # ---------------------------------------------------------------
            def aggregate(tbl, tbl_cols, last, tail=None):
                """Gather + one-hot-matmul segment sums.
                layers 0/1 (last=False): raw sums into aggT (st rows 0:h,
                ts rows h:2h).  last=True: both branches into outTL[:c_out].
                tail(blocks) is emitted right after each supergroup's
                copies so post-processing / next-layer table building
                overlaps the next supergroup's gathers."""
                for sgi, blocks in enumerate(lay.sg_blocks):
                    bufs = {}
                    for b in range(2):
                        for hf in range(2):
                            ch0 = int(lay.half_chunk_off[b, blocks[0], hf])
                            ch1 = int(lay.half_chunk_off[b, blocks[-1], hf]
                                      + lay.cap_chunks[b, blocks[-1], hf])
                            nch = ch1 - ch0
                            if nch == 0:
                                continue
                            buf = mp.tile([P, nch, tbl_cols], BF16,
                                          tag=f"msg{b}{hf}")
                            nidx = nch * P
                            nc.gpsimd.dma_gather(
                                out_ap=buf[:], in_ap=tbl[hf * HALF:, :],
                                idxs_ap=idx_sb[b, hf][:, ch0 * 8: ch1 * 8],
                                num_idxs=nidx,
                                num_idxs_reg=nidx, elem_size=tbl_cols,
                                single_packet=(nidx <= 1024),
                                queue_num=next_q())
                            bufs[b, hf] = (buf, ch0)
                    # ---- masks: one broadcast DVE op per (branch, sg) ----
                    co = {}
                    malls = {}
                    for b in range(2):
                        c0 = int(lay.chunk_off[b, blocks[0], 0])
                        c1 = int(lay.chunk_off[b, blocks[-1], 1]
                                 + lay.cap_chunks[b, blocks[-1], 1])
                        co[b] = c0
                        nch_b = c1 - c0
                        if nch_b == 0:
                            continue
                        mall = kp.tile([P, nch_b, P], BF16, tag="mask")
                        nc.vector.tensor_tensor(
                            out=mall[:],
                            in0=dt_sb[b][:, c0:c1, 0:1].to_broadcast(
                                [P, nch_b, P]),
                            in1=iota3[:, 0:1, :].to_broadcast([P, nch_b, P]),
                            op=mybir.AluOpType.is_equal)
                        malls[b] = mall

                    def mask_col(b, cc):
                        return malls[b][:, cc - co[b], :]

                    for blk in blocks:
                        nb = slice(blk * P, min((blk + 1) * P, nblk_pad))
                        if last:
                            pss = {}
                        else:
                            ps2 = psa.tile([P, P], F32, tag="agg2")
                        wrote = [False, False]
                        for b in range(2):
                            chunks = []
                            for hf in range(2):
                                for j in range(int(lay.cap_chunks[b, blk, hf])):
                                    chunks.append((hf, j))
                            if not chunks:
                                continue
                            if last:
                                ps = psa.tile([c_out, P], F32, tag="agg2")
                                pss[b] = ps
                            for ci, (hf, j) in enumerate(chunks):
                                buf, ch0 = bufs[b, hf]
                                jj = (int(lay.half_chunk_off[b, blk, hf])
                                      - ch0 + j)
                                cc = int(lay.chunk_off[b, blk, hf]) + j
                                if last:
                                    lh = buf[:, jj, 0:c_out]
                                    o = ps[:, :]
                                    tpos = None
                                else:
                                    lh = buf[:, jj, b * h:(b + 1) * h]
                                    o = ps2[b * h:(b + 1) * h, :]
                                    tpos = (0, b * h)
                                nc.tensor.matmul(
                                    o, lhsT=lh, rhs=mask_col(b, cc),
                                    start=(ci == 0),
                                    stop=(ci == len(chunks) - 1),
                                    tile_position=tpos)
                            wrote[b] = True
                        if last:
                            if wrote[0]:
                                nc.scalar.copy(outTL[:, nb], pss[0][:, :])
                            else:
                                nc.vector.memset(outTL[:, nb], 0.0)
                            if wrote[1]:
                                nc.vector.tensor_add(out=outTL[:, nb],
                                                     in0=outTL[:, nb],
                                                     in1=pss[1][:, :])
                        else:
                            for b in range(2):
                                r = slice(b * h, (b + 1) * h)
                                if wrote[b]:
                                    nc.scalar.copy(aggT[r, nb], ps2[r, :])
                                else:
                                    nc.vector.memset(aggT[r, nb], 0.0)
                    if tail is not None:
                        tail(blocks)

            # ---------------------------------------------------------------
            def post01_slice(layer, out_tile, s0, s1):
                """out_tile = relu((aggT + xwT*dinvb) * dinvb + bias)"""
                nc.vector.tensor_tensor(out=xwT[:, s0:s1], in0=xwT[:, s0:s1],
                                        in1=dinvb[:, s0:s1],
                                        op=mybir.AluOpType.mult)
                nc.vector.tensor_tensor(out=aggT[:, s0:s1], in0=aggT[:, s0:s1],
                                        in1=xwT[:, s0:s1],
                                        op=mybir.AluOpType.add)
                nc.vector.tensor_tensor(out=aggT[:, s0:s1], in0=aggT[:, s0:s1],
                                        in1=dinvb[:, s0:s1],
                                        op=mybir.AluOpType.mult)
                nc.scalar.activation(out_tile[:, s0:s1], aggT[:, s0:s1],
                                     mybir.ActivationFunctionType.Relu,
                                     bias=biasv[:, layer:layer + 1])

            def tables_slice(hsrc, w_sb, wcols, slp_rows, slp_dst, tbl_dst,
                             prescale, blocks):
                """Next-layer table build restricted to a supergroup's
                node columns (src has kch=1)."""
                t0 = blocks[0] * P
                t1 = min(blocks[-1] * P + P, nloc)
                if t1 > t0:
                    ps = psp.tile([P, 512], F32, tag="pst")
                    nc.tensor.matmul(ps[:slp_rows, :t1 - t0],
                                     lhsT=w_sb[:, :slp_rows],
                                     rhs=hsrc[:, t0:t1],
                                     start=True, stop=True)
                    nc.scalar.copy(slp_dst[:slp_rows, t0:t1],
                                   ps[:slp_rows, :t1 - t0])
                for blk in blocks:
                    nb0 = blk * P
                    nb1 = min(nb0 + P, nloc)
                    if nb1 <= nb0:
                        continue
                    nn = nb1 - nb0
                    ps2 = psp.tile([P, wcols], F32, tag="psnm")
                    nc.tensor.matmul(ps2[:nn, :], lhsT=hsrc[:, nb0:nb1],
                                     rhs=w_sb[:, :wcols],
                                     start=True, stop=True)
                    tt = wp.tile([P, wcols], BF16, tag="tblt")
                    for (c0, c1, dcol) in prescale:
                        nc.vector.tensor_scalar_mul(
                            tt[:nn, c0:c1], ps2[:nn, c0:c1],
                            dinv[:nn, dcol * nblk + blk: dcol * nblk + blk + 1])
                    nc.sync.dma_start(out=tbl_dst[nb0:nb1, 0:wcols],
                                      in_=tt[:nn, :])

            def final_slice(blocks):
                """Last-layer epilogue for a supergroup: normalization,
                bias, then per-block log_softmax and output DMA."""
                s0 = blocks[0] * P
                s1 = min(blocks[-1] * P + P, nblk_pad)
                nc.vector.tensor_tensor(out=xwTL[:, s0:s1], in0=xwTL[:, s0:s1],
                                        in1=dinvallb[:, s0:s1],
                                        op=mybir.AluOpType.mult)
                nc.vector.tensor_tensor(out=outTL[:, s0:s1], in0=outTL[:, s0:s1],
                                        in1=xwTL[:, s0:s1],
                                        op=mybir.AluOpType.add)
                nc.vector.tensor_tensor(out=outTL[:, s0:s1], in0=outTL[:, s0:s1],
                                        in1=dinvallb[:, s0:s1],
                                        op=mybir.AluOpType.mult)
                nc.scalar.activation(outTL[:, s0:s1], outTL[:, s0:s1],
                                     mybir.ActivationFunctionType.Identity,
                                     bias=biasL[:, 0:1])
                for blk in blocks:
                    nb0 = blk * P
                    nb1 = min(nb0 + P, nloc)
                    if nb1 <= nb0:
                        continue
                    nn = nb1 - nb0
                    tp = psp.tile([P, c_out], F32, tag="psnm")
                    nc.tensor.transpose(tp[:], outTL[:, nb0:nb0 + P],
                                        ident[:c_out, :c_out])
                    negmax = wp.tile([P, 1], F32, tag="negmax")
                    nc.vector.tensor_reduce(negmax[:], tp[:],
                                            axis=mybir.AxisListType.X,
                                            op=mybir.AluOpType.max, negate=True)
                    ex = wp.tile([P, c_out], F32, tag="ex")
                    nc.scalar.activation(ex[:], tp[:],
                                         mybir.ActivationFunctionType.Exp,
                                         bias=negmax[:, 0:1])
                    sume = wp.tile([P, 1], F32, tag="sume")
                    nc.vector.tensor_reduce(sume[:], ex[:],
                                            axis=mybir.AxisListType.X,
                                            op=mybir.AluOpType.add)
                    lse = wp.tile([P, 1], F32, tag="lse")
                    nc.scalar.activation(lse[:], sume[:],
                                         mybir.ActivationFunctionType.Ln)
                    fin = wp.tile([P, c_out], F32, tag="fin")
                    nc.vector.tensor_scalar(
                        out=fin[:], in0=tp[:], scalar1=negmax[:, 0:1],
                        scalar2=lse[:, 0:1], op0=mybir.AluOpType.add,
                        op1=mybir.AluOpType.subtract)
                    nc.sync.dma_start(out=out_d[nb0:nb1, :], in_=fin[:nn, :])

            def early_out(tile_ap):
                # debug escape hatch: dump a [128,c_out] sample and stop
                nc.sync.dma_start(out=out_d[0:P, :], in_=tile_ap)

            def _phases():
                if stop_at <= 1:
                    early_out(dinvb[0:P, 0:c_out])
                if nblk_pad > nloc:
                    nc.vector.memset(xwT[:, nloc:], 0.0)
                    nc.vector.memset(hT[:, nloc:], 0.0)
                # =================== layer 0 ===================
                def x_loader(k, j0, j1):
                    t = wp.tile([P, 512], BF16, tag="xk")
                    nc.sync.dma_start(
                        out=t[:, :j1 - j0],
                        in_=xT_d[:, k * nloc + j0: k * nloc + j1])
                    return t[:, :j1 - j0]

                if stop_at <= 1:
                    return
                with nc.named_scope("l0_tables"):
                    build_tables(
                        src_getter=x_loader,
                        src_kch=kch,
                        w_cols_of_k=lambda k: w0_sb[:, k * h2:(k + 1) * h2],
                        wcols=h2, slp_rows=h2, slp_dst=xwT, tbl_dst=tbl_loc[0],
                        prescale=[(0, h, 0), (h, h2, 1)])
                if stop_at <= 2:
                    early_out(xwT[0:P, 0:c_out])
                    return
                with nc.named_scope("l0_allgather"):
                    barrier()
                    nc.gpsimd.collective_compute(
                        "AllGather", mybir.AluOpType.bypass,
                        replica_groups=[core_ids],
                        ins=[tbl_loc[0][:]], outs=[tbl_full[0][:]])
                    barrier()
                if stop_at <= 3:
                    gdbg = wp.tile([P, c_out], BF16, tag="gdbg")
                    nc.sync.dma_start(out=gdbg[:], in_=tbl_full[0][0:P, 0:c_out])
                    gdbgf = wp.tile([P, c_out], F32, tag="gdbgf")
                    nc.vector.tensor_copy(gdbgf[:], gdbg[:])
                    early_out(gdbgf[:])
                    return

                # agg(l0) with interleaved post + layer-1 tables per sg
                def tail0(blocks):
                    s0 = blocks[0] * P
                    s1 = min(blocks[-1] * P + P, nblk_pad)
                    post01_slice(0, hT, s0, s1)
                    tables_slice(hT, w1_sb, h2, h2, xwT, tbl_loc[1],
                                 [(0, h, 0), (h, h2, 1)], blocks)

                with nc.named_scope("l0_agg"):
                    aggregate(tbl_full[0], h2, last=False, tail=tail0)
                if stop_at <= 5:
                    hdbg = wp.tile([P, c_out], F32, tag="hdbg")
                    nc.vector.tensor_copy(hdbg[:], hT[0:P, 0:c_out])
                    early_out(hdbg[:])
                    return

                # =================== layer 1 ===================
                with nc.named_scope("l1_allgather"):
                    barrier()
                    nc.gpsimd.collective_compute(
                        "AllGather", mybir.AluOpType.bypass,
                        replica_groups=[core_ids],
                        ins=[tbl_loc[1][:]], outs=[tbl_full[1][:]])
                    barrier()

                def tail1(blocks):
                    s0 = blocks[0] * P
                    s1 = min(blocks[-1] * P + P, nblk_pad)
                    post01_slice(1, h2T, s0, s1)
                    tables_slice(h2T, wl_sb, 128, c_out, xwTL, tblL_loc,
                                 [(0, 128, 2)], blocks)

                with nc.named_scope("l1_agg"):
                    aggregate(tbl_full[1], h2, last=False, tail=tail1)
                if stop_at <= 8:
                    hdbg2 = wp.tile([P, c_out], F32, tag="hdbg")
                    nc.vector.tensor_copy(hdbg2[:], h2T[0:P, 0:c_out])
                    early_out(hdbg2[:])
                    return

                # =================== last layer ===================
                with nc.named_scope("l2_allgather"):
                    barrier()
                    nc.gpsimd.collective_compute(
                        "AllGather", mybir.AluOpType.bypass,
                        replica_groups=[core_ids],
                        ins=[tblL_loc[:]], outs=[tblL_full[:]])
                    barrier()
                with nc.named_scope("l2_agg"):
                    aggregate(tblL_full, 128, last=True, tail=final_slice)

            _phases()

    nc.compile()
    return nc


# ----------------------------------------------------------------------------
# driver
# ----------------------------------------------------------------------------

_CACHE = {}
_RUNNER = {}


def _build_runner(nc, n_cores):
    """Persistent jitted executor (no donation; inputs stay on device)."""
    import jax
    from jax.sharding import Mesh, PartitionSpec
    from jax.experimental.shard_map import shard_map
    import concourse.mybir as mybir_
    from concourse import bass2jax
    from concourse.bass2jax import _bass_exec_p, partition_id_tensor

    bass2jax.install_neuronx_cc_hook()
    partition_name = (nc.partition_id_tensor.name
                      if nc.partition_id_tensor else None)
    in_names, out_names, out_avals, zero_outs = [], [], [], []
    for alloc in nc.m.functions[0].allocations:
        if not isinstance(alloc, mybir_.MemoryLocationSet):
            continue
        name = alloc.memorylocations[0].name
        if alloc.kind == "ExternalInput":
            if name != partition_name:
                in_names.append(name)
        elif alloc.kind == "ExternalOutput":
            out_names.append(name)
            shape = tuple(alloc.tensor_shape)
            dtype = mybir_.dt.np(alloc.dtype)
            out_avals.append(jax.core.ShapedArray(shape, dtype))
            zero_outs.append(np.zeros(shape, dtype))
    n_params = len(in_names)
    all_names = in_names + out_names
    if partition_name is not None:
        all_names.append(partition_name)

    def _body(*args):
        operands = list(args)
        if partition_name is not None:
            operands.append(partition_id_tensor())
        return tuple(_bass_exec_p.bind(
            *operands, out_avals=tuple(out_avals), in_names=tuple(all_names),
            out_names=tuple(out_names), lowering_input_output_aliases=(),
            sim_require_finite=True, sim_require_nnan=True, nc=nc))

    devices = jax.devices()[:n_cores]
    mesh = Mesh(np.asarray(devices), ("core",))
    n_out = len(out_names)
    fn = jax.jit(shard_map(_body, mesh=mesh,
                           in_specs=(PartitionSpec("core"),) * (n_params + n_out),
                           out_specs=(PartitionSpec("core"),) * n_out,
                           check_rep=False), keep_unused=True)
    return fn, in_names, out_names, out_avals, zero_outs, mesh


def _run_persistent(nc, in_maps, n_cores, key):
    import jax
    if key not in _RUNNER:
        fn, in_names, out_names, out_avals, zero_outs, mesh = \
            _build_runner(nc, n_cores)
        _RUNNER[key] = dict(fn=fn, in_names=in_names, out_names=out_names,
                            out_avals=out_avals, zero_outs=zero_outs,
                            mesh=mesh, dev_args=None)
    R = _RUNNER[key]
    concat_in = [np.concatenate([np.asarray(in_maps[c][nm])
                                 for c in range(n_cores)], axis=0)
                 for nm in R["in_names"]]
    concat_zero = [np.zeros((n_cores * z.shape[0], *z.shape[1:]), z.dtype)
                   for z in R["zero_outs"]]
    args = [jax.device_put(a) for a in concat_in + concat_zero]
    R["dev_args"] = args
    outs = R["fn"](*args)
    outs = [np.asarray(o) for o in outs]
    return {nm: outs[i].reshape(n_cores, *R["out_avals"][i].shape)
            for i, nm in enumerate(R["out_names"])}


# ---------------------------------------------------------------------------
# device timing: NTFF (neuron-profile) with difference-method fallback
# ---------------------------------------------------------------------------

def _axon_profile_hook(so_path="/opt/axon/libaxon_pjrt.so"):
    try:
        lib = ctypes.CDLL(so_path)
    except OSError:
        return None
    if not hasattr(lib, "axon_start_nrt_profile"):
        return None
    lib.axon_start_nrt_profile.argtypes = [ctypes.POINTER(ctypes.c_int64),
                                           ctypes.c_size_t]
    lib.axon_start_nrt_profile.restype = ctypes.c_int64
    lib.axon_stop_nrt_profile.argtypes = [ctypes.c_char_p]
    lib.axon_stop_nrt_profile.restype = ctypes.c_int64

    @contextlib.contextmanager
    def _hook(output_dir, device_ids):
        import jax
        jax.devices()
        if device_ids:
            ids = (ctypes.c_int64 * len(device_ids))(*device_ids)
            rc = lib.axon_start_nrt_profile(ids, len(device_ids))
        else:
            rc = lib.axon_start_nrt_profile(None, 0)
        if rc != 0:
            raise RuntimeError(f"axon_start_nrt_profile rc={rc}")
        try:
            yield
        finally:
            n = lib.axon_stop_nrt_profile(str(output_dir).encode())
            if n <= 0:
                print(f"profile capture wrote {n} files", file=sys.stderr)

    return _hook


def _ntff_exec_ns(key, devices=(0,)):
    """Profile one warm execution; return max NEFF device span in ns."""
    import jax
    hook = _axon_profile_hook()
    if hook is None:
        return None
    R = _RUNNER[key]
    fn, args = R["fn"], R["dev_args"]
    o = fn(*args)
    jax.block_until_ready(o)
    tmpdir = tempfile.mkdtemp(prefix="ntff_timing_")
    with hook(tmpdir, list(devices)):
        o = fn(*args)
        jax.block_until_ready(o)
    ntffs = sorted(glob.glob(os.path.join(tmpdir, "*_body*device*.ntff")))
    if not ntffs:
        return None
    times = []
    for i, ntff in enumerate(ntffs):
        # pair the ntff with its own executable's neff by name prefix
        prefix = os.path.basename(ntff).split("-device")[0]
        neffs = glob.glob(os.path.join(tmpdir, prefix + ".neff"))
        if not neffs:
            continue
        outj = os.path.join(tmpdir, f"prof_{i}.json")
        try:
            subprocess.run(
                ["neuron-profile", "view", "-n", neffs[0], "-s", ntff,
                 "--output-format=json", "--output-file", outj,
                 "--ignore-nc-buf-usage"],
                check=True, capture_output=True,
                env=dict(os.environ, NEURON_PROFILE_DBG_OUTPUT="2"))
            with open(outj) as f:
                j = json.load(f)
            times.append(float(j["summary"][0]["total_time"]) * 1e9)
        except Exception as e:
            print("neuron-profile failed:", e, file=sys.stderr)
    return max(times) if times else None


def time_device(inputs, iters=6, cfg=None):
    """Device execution time in ns.

    Primary: neuron-profile (NTFF) span of one warm on-device execution.
    Fallback: marginal per-execution wall time via the difference method."""
    import jax, time as _t
    cfg = cfg or FULL_CFG
    key = _ensure_built(cfg, inputs)
    if os.environ.get("BASS_TIME_MODE", "ntff") == "ntff":
        try:
            ns = _ntff_exec_ns(key)
        except Exception as e:
            print("ntff timing failed:", e, file=sys.stderr)
            ns = None
        if ns is not None:
            return ns
    R = _RUNNER[key]
    fn, args = R["fn"], R["dev_args"]
    o = fn(*args); jax.block_until_ready(o)
    t0 = _t.time()
    o = fn(*args); jax.block_until_ready(o)
    t1 = _t.time()
    base = t1 - t0
    t0 = _t.time()
    for _ in range(1 + iters):
        o = fn(*args)
    jax.block_until_ready(o)
    t1 = _t.time()
    per = (t1 - t0 - base) / iters
    return per * 1e9


def _ensure_built(cfg, inputs):
    weights = {k: np.asarray(inputs[k]) for k in
               ("W_st0", "b_st0", "W_ts0", "b_ts0", "W_st1", "b_st1",
                "W_ts1", "b_ts1", "W_last", "b_last")}
    lay, in_maps = host_prep(cfg, inputs["x"], inputs["edge_index"],
                             inputs["is_reversed"])
    wmap = host_prep_weights(cfg, **weights)
    for m in in_maps:
        m.update(wmap)
    sig = (lay.signature(), 99, "full")
    if sig not in _CACHE:
        _CACHE[sig] = build_program(cfg, lay, stop_at=99)
    nc = _CACHE[sig]
    key = str(sig)
    _run_persistent(nc, in_maps, cfg["n_cores"], key)
    return key


def run(cfg, x, edge_index, is_reversed, weights, use_sim=False, stop_at=99,
        agg_mode="full"):
    lay, in_maps = host_prep(cfg, x, edge_index, is_reversed)
    wmap = host_prep_weights(cfg, **weights)
    for m in in_maps:
        m.update(wmap)

    sig = (lay.signature(), stop_at, agg_mode)
    if sig in _CACHE:
        nc = _CACHE[sig]
    else:
        nc = build_program(cfg, lay, stop_at=stop_at)
        _CACHE[sig] = nc

    n_cores = cfg["n_cores"]
    if use_sim:
        import concourse.bass_interp as bass_interp
        sim = bass_interp.MultiCoreSim(nc, n_cores, require_finite=False,
                                       require_nnan=False)
        for c in range(n_cores):
            for k, v in in_maps[c].items():
                sim.cores[c].tensor(k)[:] = v
        sim.simulate()
        outs = [np.array(sim.cores[c].tensor("out")) for c in range(n_cores)]
    else:
        key = str(sig)
        res = _run_persistent(nc, in_maps, n_cores, key)
        outs = list(res["out"])
    return np.concatenate(outs, axis=0)


def kernel(x, edge_index, is_reversed, W_st0, b_st0, W_ts0, b_ts0,
           W_st1, b_st1, W_ts1, b_ts1, W_last, b_last):
    cfg = FULL_CFG
    weights = dict(W_st0=W_st0, b_st0=b_st0, W_ts0=W_ts0, b_ts0=b_ts0,
                   W_st1=W_st1, b_st1=b_st1, W_ts1=W_ts1, b_ts1=b_ts1,
                   W_last=W_last, b_last=b_last)
    out = run(cfg, x, edge_index, is_reversed, weights)
    return out.astype(np.float32)


# revision 33
# speedup vs baseline: 1.2734x; 1.0860x over previous
"""Trainium2 Bass kernel for nn_BiModel (2-layer bidirectional GCN).

Distribution over 8 NeuronCores: nodes sharded 6250/core, edge lists
partitioned by destination core and sorted by (branch, dst-block,
src-half).  Per GCN layer each core computes the transformed features
for its own nodes (prescaled by dinv[src]), AllGathers the bf16
message table, dma_gathers the src rows of its edges and segment-sums
them with one-hot matmuls on the tensor engine (PSUM accumulation per
128-node destination block).  Host work is limited to sharding/layout
prep (transpose/pad, edge partition+sort, degree counts, gather index
tensors); all floating-point math on tensor data runs on device.

v2 perf changes vs baseline:
- exact data-driven chunk capacities (no floors): ~16% fewer gather
  descriptors / masks / matmuls
- dma_gather descriptor generation round-robins 4 SWDGE queues (2-way
  parallel Q7 generation measured on HW)
- one-hot masks built in ONE broadcast-AP tensor_tensor per
  (branch-pair, supergroup) instead of one DVE op per chunk
- gathers read the AllGather output directly (no DRAM->DRAM copy)
- message buffers double-buffered across supergroups
"""

import contextlib
import ctypes
import glob
import json
import os
import subprocess
import sys
import tempfile
import types

import numpy as np

import concourse.bass as bass
import concourse.bacc as bacc
import concourse.mybir as mybir
import concourse.tile as tile
from concourse.bass_utils import run_bass_kernel_spmd
from concourse.masks import make_identity

import ml_dtypes

P = 128
F32 = mybir.dt.float32
BF16 = mybir.dt.bfloat16
I16 = mybir.dt.int16
I32 = mybir.dt.int32
HALF = 32768

FULL_CFG = dict(n=50000, e=800000, f_in=500, h=64, c_out=16, n_cores=8,
                sb_blocks=4, cap_floor=None, nqueues=4)


def cdiv(a, b):
    return (a + b - 1) // b


# ----------------------------------------------------------------------------
# host-side layout / preprocessing
# ----------------------------------------------------------------------------

class Layout:
    """Compile-time chunk layout, shared by all cores (uniform SPMD
    program).  Group = (branch, dst-block, src-half); per-group chunk
    capacity = max edge count over cores, rounded up to 128."""

    def __init__(self, cfg, group_counts):
        # group_counts: [n_cores, 2, nblk, 2]
        self.cfg = cfg
        self.nloc = cfg["n"] // cfg["n_cores"]
        self.nblk = cdiv(self.nloc, P)
        cap = group_counts.max(axis=0)                    # [2, nblk, 2]
        self.cap_chunks = cdiv(cap, P)                    # may be 0
        floor = cfg.get("cap_floor")
        if floor is not None:
            flo, fhi = floor
            self.cap_chunks = np.maximum(
                self.cap_chunks,
                np.array([[[flo, fhi]]], np.int64))
        # branch-major chunk stream (for dstloc): (blk, half) order
        self.chunk_off = np.zeros((2, self.nblk, 2), np.int64)
        for b in range(2):
            off = 0
            for blk in range(self.nblk):
                for hf in range(2):
                    self.chunk_off[b, blk, hf] = off
                    off += self.cap_chunks[b, blk, hf]
        self.nchunks_br = self.chunk_off[:, -1, 1] + self.cap_chunks[:, -1, 1]
        # per-(branch, half) gather stream: blk order
        self.half_chunk_off = np.zeros((2, self.nblk, 2), np.int64)
        self.nchunks_bh = np.zeros((2, 2), np.int64)
        for b in range(2):
            for hf in range(2):
                off = 0
                for blk in range(self.nblk):
                    self.half_chunk_off[b, blk, hf] = off
                    off += self.cap_chunks[b, blk, hf]
                self.nchunks_bh[b, hf] = off
        sb = cfg["sb_blocks"]
        self.sg_blocks = [list(range(g * sb, min((g + 1) * sb, self.nblk)))
                          for g in range(cdiv(self.nblk, sb))]

    def signature(self):
        return (tuple(self.cap_chunks.reshape(-1).tolist()),
                tuple(sorted((k, str(v)) for k, v in self.cfg.items())))


def _wrap_idx16(idx, n_pad):
    buf = np.zeros(n_pad, np.int16)
    buf[: len(idx)] = idx.astype(np.int16)
    w = buf.reshape(n_pad // 16, 16).T            # [16, n/16]
    return np.ascontiguousarray(np.tile(w, (8, 1)))  # [128, n/16]


def host_prep(cfg, x, edge_index, is_reversed):
    n, f_in = cfg["n"], cfg["f_in"]
    n_cores = cfg["n_cores"]
    nloc = n // n_cores
    f_pad = cdiv(f_in, P) * P

    src = np.asarray(edge_index[0], np.int64)
    dst = np.asarray(edge_index[1], np.int64)
    rev = np.asarray(is_reversed).astype(bool)

    core = dst // nloc
    dl = dst % nloc
    blk = dl // P
    sblk = (dl % P).astype(np.float32)
    branch = rev.astype(np.int64)
    hf = (src >= HALF).astype(np.int64)

    nblk = cdiv(nloc, P)
    key = ((core * 2 + branch) * nblk + blk) * 2 + hf
    order = np.argsort(key, kind="stable")
    counts = np.bincount(key[order], minlength=n_cores * 2 * nblk * 2)
    counts = counts.reshape(n_cores, 2, nblk, 2)
    lay = Layout(cfg, counts)

    deg = np.zeros((2, n), np.float32)
    np.add.at(deg[0], dst[~rev], 1.0)
    np.add.at(deg[1], dst[rev], 1.0)

    xT = np.zeros((f_pad, n), ml_dtypes.bfloat16)
    xT[:f_in] = np.asarray(x, np.float32).T
    kch = f_pad // P

    src_s = src[order]
    sblk_s = sblk[order]
    gs = np.concatenate([[0], np.cumsum(counts.reshape(-1))])[:-1]
    gs = gs.reshape(n_cores, 2, nblk, 2)

    nblk_pad = nblk * P
    in_maps = []
    for c in range(n_cores):
        xc = xT[:, c * nloc:(c + 1) * nloc].reshape(kch, P, nloc)
        # tile-major x layout: one contiguous [P, kch*L] region per 512-col
        # tile so layer-0 loads take a single DMA each
        parts = []
        for t0 in range(0, nloc, 512):
            t1 = min(t0 + 512, nloc)
            parts.append(np.ascontiguousarray(
                xc[:, :, t0:t1].transpose(1, 0, 2).reshape(P, kch * (t1 - t0))))
        m = {"xT": np.concatenate(parts, axis=1)}
        degs = np.ones((P, 2 * nblk), np.float32)
        for b in range(2):
            dloc = np.ones(nblk_pad, np.float32)
            dloc[:nloc] = deg[b, c * nloc:(c + 1) * nloc]
            degs[:, b * nblk:(b + 1) * nblk] = dloc.reshape(nblk, P).T
        m["degs"] = degs
        dflat = np.zeros((1, 3 * nblk_pad), np.float32)
        dflat[0, :nloc] = deg[0, c * nloc:(c + 1) * nloc]
        dflat[0, nblk_pad:nblk_pad + nloc] = deg[1, c * nloc:(c + 1) * nloc]
        dflat[0, 2 * nblk_pad:2 * nblk_pad + nloc] = (
            deg[0, c * nloc:(c + 1) * nloc] + deg[1, c * nloc:(c + 1) * nloc])
        m["degs_flat"] = dflat
        for b in range(2):
            ncol = max(int(lay.nchunks_br[b]), 1)
            dst_cols = np.full((P, ncol), -1.0, np.float32)
            for hf_ in range(2):
                tot = max(int(lay.nchunks_bh[b, hf_]), 1) * P
                idx_stream = np.zeros(tot, np.int16)
                for blk_ in range(lay.nblk):
                    cnt = int(counts[c, b, blk_, hf_])
                    s0 = int(gs[c, b, blk_, hf_])
                    ho = int(lay.half_chunk_off[b, blk_, hf_]) * P
                    idx_stream[ho:ho + cnt] = src_s[s0:s0 + cnt] - hf_ * HALF
                    co = int(lay.chunk_off[b, blk_, hf_])
                    ce = int(lay.cap_chunks[b, blk_, hf_])
                    if ce:
                        dv = np.full(ce * P, -1.0, np.float32)
                        dv[:cnt] = sblk_s[s0:s0 + cnt]
                        dst_cols[:, co:co + ce] = dv.reshape(ce, P).T
                m[f"idx_b{b}h{hf_}"] = _wrap_idx16(idx_stream, tot)
            m[f"dstloc_b{b}"] = dst_cols.astype(ml_dtypes.bfloat16)
        in_maps.append(m)
    return lay, in_maps


def host_prep_weights(cfg, W_st0, b_st0, W_ts0, b_ts0, W_st1, b_st1,
                      W_ts1, b_ts1, W_last, b_last):
    f_in, h, c_out = cfg["f_in"], cfg["h"], cfg["c_out"]
    f_pad = cdiv(f_in, P) * P
    W0 = np.zeros((f_pad, 2 * h), np.float32)
    W0[:f_in, :h] = W_st0
    W0[:f_in, h:] = W_ts0
    kch = f_pad // P
    W0 = np.ascontiguousarray(
        W0.reshape(kch, P, 2 * h).transpose(1, 0, 2).reshape(P, kch * 2 * h))
    W1 = np.concatenate([W_st1, W_ts1], axis=1).astype(np.float32)
    WL = np.zeros((2 * h, 128), np.float32)
    WL[:, :c_out] = W_last
    bias01 = np.stack([np.concatenate([b_st0, b_ts0]),
                       np.concatenate([b_st1, b_ts1])], axis=1).astype(np.float32)
    return dict(W0=W0.astype(ml_dtypes.bfloat16),
                W1=W1.astype(ml_dtypes.bfloat16),
                WL=WL.astype(ml_dtypes.bfloat16), bias01=bias01,
                b_last=np.asarray(b_last, np.float32).reshape(c_out, 1))


# ----------------------------------------------------------------------------
# device program
# ----------------------------------------------------------------------------

def build_program(cfg, lay, stop_at=99, agg_mode="full"):
    n, f_in = cfg["n"], cfg["f_in"]
    h, c_out = cfg["h"], cfg["c_out"]
    n_cores = cfg["n_cores"]
    nqueues = cfg.get("nqueues", 4)
    nloc = n // n_cores
    nblk = lay.nblk
    nblk_pad = nblk * P
    f_pad = cdiv(f_in, P) * P
    kch = f_pad // P
    h2 = 2 * h
    core_ids = list(range(n_cores))

    nc = bacc.Bacc("TRN2", target_bir_lowering=False, debug=False,
                   num_devices=n_cores, num_swdge_queues=nqueues)

    xT_d = nc.declare_dram_parameter("xT", [P, kch * nloc], BF16, isOutput=False)
    degs_d = nc.declare_dram_parameter("degs", [P, 2 * nblk], F32, isOutput=False)
    degsf_d = nc.declare_dram_parameter("degs_flat", [1, 3 * (cdiv(nloc, P) * P)],
                                        F32, isOutput=False)
    W0_d = nc.declare_dram_parameter("W0", [P, kch * h2], BF16, isOutput=False)
    W1_d = nc.declare_dram_parameter("W1", [h2, h2], BF16, isOutput=False)
    WL_d = nc.declare_dram_parameter("WL", [h2, 128], BF16, isOutput=False)
    bias01_d = nc.declare_dram_parameter("bias01", [h2, 2], F32, isOutput=False)
    b_last_d = nc.declare_dram_parameter("b_last", [c_out, 1], F32, isOutput=False)
    idx_d = {}
    for b in range(2):
        for hf in range(2):
            w = max(int(lay.nchunks_bh[b, hf]), 1) * 8
            idx_d[b, hf] = nc.declare_dram_parameter(
                f"idx_b{b}h{hf}", [P, w], I16, isOutput=False)
    dstloc_d = [nc.declare_dram_parameter(
        f"dstloc_b{b}", [P, max(int(lay.nchunks_br[b]), 1)], BF16,
        isOutput=False) for b in range(2)]
    out_d = nc.declare_dram_parameter("out", [nloc, c_out], F32, isOutput=True)

    tbl_loc = [nc.dram_tensor(f"tbl_loc{i}", [nloc, h2], BF16) for i in range(2)]
    tbl_full = [nc.dram_tensor(f"tbl_full{i}", [n, h2], BF16,
                               addr_space="Shared") for i in range(2)]
    tblL_loc = nc.dram_tensor("tblL_loc", [nloc, 128], BF16)
    tblL_full = nc.dram_tensor("tblL_full", [n, 128], BF16, addr_space="Shared")

    qctr = [0]

    def next_q():
        # rotate the (b,hf)->queue mapping every supergroup so the big
        # hf0 streams don't always land on the same queues
        q = (qctr[0] + qctr[0] // nqueues) % nqueues
        qctr[0] += 1
        return q

    with tile.TileContext(nc) as tc:
        with (
            tc.tile_pool(name="persist", bufs=1) as pp,
            tc.tile_pool(name="work", bufs=2) as wp,
            tc.tile_pool(name="msg", bufs=3) as mp,
            tc.tile_pool(name="mask", bufs=2) as kp,
            tc.tile_pool(name="psum", bufs=2, space="PSUM") as psp,
            tc.tile_pool(name="psagg", bufs=3, space="PSUM") as psa,
        ):
            # ---------- constants ----------
            iota_i = wp.tile([P, P], I32, tag="ioi")
            nc.gpsimd.iota(iota_i[:], pattern=[[1, P]], base=0,
                           channel_multiplier=0)
            iota3 = pp.tile([P, 1, P], BF16, tag="io3")
            nc.vector.tensor_copy(iota3[:, 0, :], iota_i[:])
            ident = pp.tile([P, P], F32, tag="ident")
            make_identity(nc, ident[:])
            biasv = pp.tile([P, 2], F32, tag="biasv")
            nc.sync.dma_start(out=biasv[:], in_=bias01_d[:, :])
            biasL = pp.tile([c_out, 1], F32, tag="biasL")
            nc.sync.dma_start(out=biasL[:], in_=b_last_d[:, :])

            def barrier():
                tc.strict_bb_all_engine_barrier()

            # ---------- degrees -> dinv [128, 3*nblk] (st|ts|all) ----------
            deg_sb = wp.tile([P, 2 * nblk], F32, tag="degsb")
            nc.sync.dma_start(out=deg_sb[:], in_=degs_d[:, :])
            dtmp = wp.tile([P, 3 * nblk], F32, tag="dtmp")
            nc.vector.tensor_tensor(out=dtmp[:, 2 * nblk:],
                                    in0=deg_sb[:, :nblk], in1=deg_sb[:, nblk:],
                                    op=mybir.AluOpType.add)
            nc.vector.tensor_copy(dtmp[:, :2 * nblk], deg_sb[:])
            nc.vector.tensor_scalar_add(dtmp[:], dtmp[:], 1.0)
            dsq = wp.tile([P, 3 * nblk], F32, tag="dsq")
            nc.scalar.sqrt(dsq[:], dtmp[:])
            dinv = pp.tile([P, 3 * nblk], F32, tag="dinv")
            nc.vector.reciprocal(dinv[:], dsq[:])

            # dinvb/dinvallb: broadcast raw degrees across partitions via
            # K=1 matmul, then Rsqrt(deg + 1) straight out of PSUM
            ones_row = pp.tile([1, P], F32, tag="ones_row")
            nc.vector.memset(ones_row[:], 1.0)
            dinvb = pp.tile([P, nblk_pad], BF16, tag="dinvb")
            dinvallb = pp.tile([c_out, nblk_pad], BF16, tag="dinvallb")
            NTB = 512
            for t0 in range(0, nblk_pad, NTB):
                t1 = min(t0 + NTB, nblk_pad)
                dfs = wp.tile([1, 3 * NTB], F32, tag="dfs")
                for i in range(3):
                    nc.sync.dma_start(
                        out=dfs[0:1, i * NTB: i * NTB + t1 - t0],
                        in_=degsf_d[0:1, i * nblk_pad + t0: i * nblk_pad + t1])
                bps = psp.tile([P, NTB], F32, tag="pst")
                nc.tensor.matmul(bps[0:h, :t1 - t0], lhsT=ones_row[0:1, 0:h],
                                 rhs=dfs[0:1, 0:t1 - t0],
                                 start=True, stop=True)
                nc.tensor.matmul(bps[h:h2, :t1 - t0], lhsT=ones_row[0:1, 0:h],
                                 rhs=dfs[0:1, NTB:NTB + t1 - t0],
                                 start=True, stop=True, tile_position=(0, h))
                dsq1 = wp.tile([P, NTB], F32, tag="dsq1")
                nc.scalar.activation(dsq1[:, :t1 - t0], bps[:, :t1 - t0],
                                     mybir.ActivationFunctionType.Sqrt,
                                     bias=1.0)
                with nc.allow_low_precision(reason="dinv broadcast in bf16"):
                    nc.vector.reciprocal(dinvb[:, t0:t1], dsq1[:, :t1 - t0])
                bps2 = psp.tile([c_out, NTB], F32, tag="psnm")
                nc.tensor.matmul(bps2[:, :t1 - t0], lhsT=ones_row[0:1, 0:c_out],
                                 rhs=dfs[0:1, 2 * NTB:2 * NTB + t1 - t0],
                                 start=True, stop=True)
                dsq2 = wp.tile([c_out, NTB], F32, tag="dsq2")
                nc.scalar.activation(dsq2[:, :t1 - t0], bps2[:, :t1 - t0],
                                     mybir.ActivationFunctionType.Sqrt,
                                     bias=1.0)
                with nc.allow_low_precision(reason="dinv broadcast in bf16"):
                    nc.vector.reciprocal(dinvallb[:, t0:t1], dsq2[:, :t1 - t0])

            # ---------- weights ----------
            w0_sb = pp.tile([P, kch * h2], BF16, tag="w0")
            nc.sync.dma_start(out=w0_sb[:], in_=W0_d[:, :])
            w1_sb = pp.tile([P, h2], BF16, tag="w1")
            nc.sync.dma_start(out=w1_sb[:], in_=W1_d[:, :])
            wl_sb = pp.tile([P, 128], BF16, tag="wl")
            nc.sync.dma_start(out=wl_sb[:], in_=WL_d[:, :])

            # ---------- edge metadata, loaded once (shared by all layers) --
            idx_sb = {}
            for b in range(2):
                for hf in range(2):
                    w = max(int(lay.nchunks_bh[b, hf]), 1) * 8
                    t = pp.tile([P, w], I16, tag=f"idxpre{b}{hf}")
                    nc.sync.dma_start(out=t[:], in_=idx_d[b, hf][:, :])
                    idx_sb[b, hf] = t
            dt_sb = []
            for b in range(2):
                ncol = max(int(lay.nchunks_br[b]), 1)
                t = pp.tile([P, ncol, 1], BF16, tag=f"dtpre{b}")
                nc.sync.dma_start(out=t[:, :, 0], in_=dstloc_d[b][:, :])
                dt_sb.append(t)

            # ---------- state ----------
            hT = pp.tile([P, nblk_pad], BF16, tag="hT")
            h2T = hT  # layer-1 output reuses the buffer (dead after use)
            xwT = pp.tile([P, nblk_pad], BF16, tag="xwT")
            aggT = pp.tile([P, nblk_pad], F32, tag="aggT")
            # last-layer [16,*] views over tiles that are dead by then
            xwTL = xwT[0:c_out, :]
            outTL = aggT[0:c_out, :]

            # ---------------------------------------------------------------
            def build_tables0():
                """Layer-0 tables from tile-major x: per 512-node tile one
                DMA load, transposed matmuls into xwT, then node-major
                matmuls + prescale + table write per 128-node block."""
                NT = 512
                off = 0
                for t0 in range(0, nloc, NT):
                    t1 = min(t0 + NT, nloc)
                    L = t1 - t0
                    xt = wp.tile([P, kch * NT], BF16, tag="xk")
                    nc.sync.dma_start(out=xt[:, :kch * L],
                                      in_=xT_d[:, off:off + kch * L])
                    off += kch * L
                    ps = psp.tile([P, NT], F32, tag="pst")
                    for k in range(kch):
                        nc.tensor.matmul(
                            ps[:h2, :L],
                            lhsT=w0_sb[:, k * h2:(k + 1) * h2],
                            rhs=xt[:, k * L:(k + 1) * L],
                            start=(k == 0), stop=(k == kch - 1))
                    nc.scalar.copy(xwT[:h2, t0:t1], ps[:h2, :L])
                    for blk in range(t0 // P, cdiv(t1, P)):
                        nb0 = blk * P
                        nb1 = min(nb0 + P, nloc)
                        nn = nb1 - nb0
                        ps2 = psp.tile([P, h2], F32, tag="psnm")
                        for k in range(kch):
                            nc.tensor.matmul(
                                ps2[:nn, :],
                                lhsT=xt[:, k * L + nb0 - t0: k * L + nb1 - t0],
                                rhs=w0_sb[:, k * h2:(k + 1) * h2],
                                start=(k == 0), stop=(k == kch - 1))
                        tt = wp.tile([P, h2], BF16, tag="tblt")
                        for (c0, c1, dcol) in ((0, h, 0), (h, h2, 1)):
                            nc.vector.tensor_scalar_mul(
                                tt[:nn, c0:c1], ps2[:nn, c0:c1],
                                dinv[:nn, dcol * nblk + blk:
                                     dcol * nblk + blk + 1])
                        nc.sync.dma_start(out=tbl_loc[0][nb0:nb1, 0:h2],
                                          in_=tt[:nn, :])

            # ---------------------------------------------------------------
            def aggregate(tbl, tbl_cols, last, tail=None):
                """Gather + one-hot-matmul segment sums.
                layers 0/1 (last=False): raw sums into aggT (st rows 0:h,
                ts rows h:2h).  last=True: both branches into outTL[:c_out].
                tail(blocks) is emitted right after each supergroup's
                copies so post-processing / next-layer table building
                overlaps the next supergroup's gathers."""
                for sgi, blocks in enumerate(lay.sg_blocks):
                    bufs = {}
                    for b in range(2):
                        for hf in range(2):
                            ch0 = int(lay.half_chunk_off[b, blocks[0], hf])
                            ch1 = int(lay.half_chunk_off[b, blocks[-1], hf]
                                      + lay.cap_chunks[b, blocks[-1], hf])
                            nch = ch1 - ch0
                            if nch == 0:
                                continue
                            buf = mp.tile([P, nch, tbl_cols], BF16,
                                          tag=f"msg{b}{hf}")
                            nidx = nch * P
                            nc.gpsimd.dma_gather(
                                out_ap=buf[:], in_ap=tbl[hf * HALF:, :],
                                idxs_ap=idx_sb[b, hf][:, ch0 * 8: ch1 * 8],
                                num_idxs=nidx,
                                num_idxs_reg=nidx, elem_size=tbl_cols,
                                single_packet=(nidx <= 1024),
                                queue_num=next_q())
                            bufs[b, hf] = (buf, ch0)
                    # ---- masks: one broadcast DVE op per (branch, sg) ----
                    co = {}
                    malls = {}
                    for b in range(2):
                        c0 = int(lay.chunk_off[b, blocks[0], 0])
                        c1 = int(lay.chunk_off[b, blocks[-1], 1]
                                 + lay.cap_chunks[b, blocks[-1], 1])
                        co[b] = c0
                        nch_b = c1 - c0
                        if nch_b == 0:
                            continue
                        mall = kp.tile([P, nch_b, P], BF16, tag="mask")
                        nc.vector.tensor_tensor(
                            out=mall[:],
                            in0=dt_sb[b][:, c0:c1, 0:1].to_broadcast(
                                [P, nch_b, P]),
                            in1=iota3[:, 0:1, :].to_broadcast([P, nch_b, P]),
                            op=mybir.AluOpType.is_equal)
                        malls[b] = mall

                    def mask_col(b, cc):
                        return malls[b][:, cc - co[b], :]

                    for blk in blocks:
                        nb = slice(blk * P, min((blk + 1) * P, nblk_pad))
                        if last:
                            pss = {}
                        else:
                            ps2 = psa.tile([P, P], F32, tag="agg2")
                        wrote = [False, False]
                        for b in range(2):
                            chunks = []
                            for hf in range(2):
                                for j in range(int(lay.cap_chunks[b, blk, hf])):
                                    chunks.append((hf, j))
                            if not chunks:
                                continue
                            if last:
                                ps = psa.tile([c_out, P], F32, tag="agg2")
                                pss[b] = ps
                            for ci, (hf, j) in enumerate(chunks):
                                buf, ch0 = bufs[b, hf]
                                jj = (int(lay.half_chunk_off[b, blk, hf])
                                      - ch0 + j)
                                cc = int(lay.chunk_off[b, blk, hf]) + j
                                if last:
                                    lh = buf[:, jj, 0:c_out]
                                    o = ps[:, :]
                                    tpos = None
                                else:
                                    lh = buf[:, jj, b * h:(b + 1) * h]
                                    o = ps2[b * h:(b + 1) * h, :]
                                    tpos = (0, b * h)
                                nc.tensor.matmul(
                                    o, lhsT=lh, rhs=mask_col(b, cc),
                                    start=(ci == 0),
                                    stop=(ci == len(chunks) - 1),
                                    tile_position=tpos)
                            wrote[b] = True
                        if last:
                            if wrote[0]:
                                nc.scalar.copy(outTL[:, nb], pss[0][:, :])
                            else:
                                nc.vector.memset(outTL[:, nb], 0.0)
                            if wrote[1]:
                                nc.vector.tensor_add(out=outTL[:, nb],
                                                     in0=outTL[:, nb],
                                                     in1=pss[1][:, :])
                        else:
                            for b in range(2):
                                r = slice(b * h, (b + 1) * h)
                                if wrote[b]:
                                    nc.scalar.copy(aggT[r, nb], ps2[r, :])
                                else:
                                    nc.vector.memset(aggT[r, nb], 0.0)
                    if tail is not None:
                        tail(blocks)

            # ---------------------------------------------------------------
            def post01_slice(layer, out_tile, s0, s1):
                """out_tile = relu((aggT + xwT*dinvb) * dinvb + bias)"""
                nc.vector.tensor_tensor(out=xwT[:, s0:s1], in0=xwT[:, s0:s1],
                                        in1=dinvb[:, s0:s1],
                                        op=mybir.AluOpType.mult)
                nc.vector.tensor_tensor(out=aggT[:, s0:s1], in0=aggT[:, s0:s1],
                                        in1=xwT[:, s0:s1],
                                        op=mybir.AluOpType.add)
                nc.vector.tensor_tensor(out=aggT[:, s0:s1], in0=aggT[:, s0:s1],
                                        in1=dinvb[:, s0:s1],
                                        op=mybir.AluOpType.mult)
                nc.scalar.activation(out_tile[:, s0:s1], aggT[:, s0:s1],
                                     mybir.ActivationFunctionType.Relu,
                                     bias=biasv[:, layer:layer + 1])

            def tables_slice(hsrc, w_sb, wcols, slp_rows, slp_dst, tbl_dst,
                             prescale, blocks):
                """Next-layer table build restricted to a supergroup's
                node columns (src has kch=1)."""
                t0 = blocks[0] * P
                t1 = min(blocks[-1] * P + P, nloc)
                if t1 > t0:
                    ps = psp.tile([P, 512], F32, tag="pst")
                    nc.tensor.matmul(ps[:slp_rows, :t1 - t0],
                                     lhsT=w_sb[:, :slp_rows],
                                     rhs=hsrc[:, t0:t1],
                                     start=True, stop=True)
                    nc.scalar.copy(slp_dst[:slp_rows, t0:t1],
                                   ps[:slp_rows, :t1 - t0])
                for blk in blocks:
                    nb0 = blk * P
                    nb1 = min(nb0 + P, nloc)
                    if nb1 <= nb0:
                        continue
                    nn = nb1 - nb0
                    ps2 = psp.tile([P, wcols], F32, tag="psnm")
                    nc.tensor.matmul(ps2[:nn, :], lhsT=hsrc[:, nb0:nb1],
                                     rhs=w_sb[:, :wcols],
                                     start=True, stop=True)
                    tt = wp.tile([P, wcols], BF16, tag="tblt")
                    for (c0, c1, dcol) in prescale:
                        nc.vector.tensor_scalar_mul(
                            tt[:nn, c0:c1], ps2[:nn, c0:c1],
                            dinv[:nn, dcol * nblk + blk: dcol * nblk + blk + 1])
                    nc.sync.dma_start(out=tbl_dst[nb0:nb1, 0:wcols],
                                      in_=tt[:nn, :])

            def final_slice(blocks):
                """Last-layer epilogue for a supergroup: normalization,
                bias, then per-block log_softmax and output DMA."""
                s0 = blocks[0] * P
                s1 = min(blocks[-1] * P + P, nblk_pad)
                nc.vector.tensor_tensor(out=xwTL[:, s0:s1], in0=xwTL[:, s0:s1],
                                        in1=dinvallb[:, s0:s1],
                                        op=mybir.AluOpType.mult)
                nc.vector.tensor_tensor(out=outTL[:, s0:s1], in0=outTL[:, s0:s1],
                                        in1=xwTL[:, s0:s1],
                                        op=mybir.AluOpType.add)
                nc.vector.tensor_tensor(out=outTL[:, s0:s1], in0=outTL[:, s0:s1],
                                        in1=dinvallb[:, s0:s1],
                                        op=mybir.AluOpType.mult)
                nc.scalar.activation(outTL[:, s0:s1], outTL[:, s0:s1],
                                     mybir.ActivationFunctionType.Identity,
                                     bias=biasL[:, 0:1])
                for blk in blocks:
                    nb0 = blk * P
                    nb1 = min(nb0 + P, nloc)
                    if nb1 <= nb0:
                        continue
                    nn = nb1 - nb0
                    tp = psp.tile([P, c_out], F32, tag="psnm")
                    nc.tensor.transpose(tp[:], outTL[:, nb0:nb0 + P],
                                        ident[:c_out, :c_out])
                    negmax = wp.tile([P, 1], F32, tag="negmax")
                    nc.vector.tensor_reduce(negmax[:], tp[:],
                                            axis=mybir.AxisListType.X,
                                            op=mybir.AluOpType.max, negate=True)
                    ex = wp.tile([P, c_out], F32, tag="ex")
                    nc.scalar.activation(ex[:], tp[:],
                                         mybir.ActivationFunctionType.Exp,
                                         bias=negmax[:, 0:1])
                    sume = wp.tile([P, 1], F32, tag="sume")
                    nc.vector.tensor_reduce(sume[:], ex[:],
                                            axis=mybir.AxisListType.X,
                                            op=mybir.AluOpType.add)
                    lse = wp.tile([P, 1], F32, tag="lse")
                    nc.scalar.activation(lse[:], sume[:],
                                         mybir.ActivationFunctionType.Ln)
                    fin = wp.tile([P, c_out], F32, tag="fin")
                    nc.vector.tensor_scalar(
                        out=fin[:], in0=tp[:], scalar1=negmax[:, 0:1],
                        scalar2=lse[:, 0:1], op0=mybir.AluOpType.add,
                        op1=mybir.AluOpType.subtract)
                    nc.sync.dma_start(out=out_d[nb0:nb1, :], in_=fin[:nn, :])

            def early_out(tile_ap):
                # debug escape hatch: dump a [128,c_out] sample and stop
                nc.sync.dma_start(out=out_d[0:P, :], in_=tile_ap)

            def _phases():
                if stop_at <= 1:
                    early_out(dinvb[0:P, 0:c_out])
                if nblk_pad > nloc:
                    nc.vector.memset(xwT[:, nloc:], 0.0)
                    nc.vector.memset(hT[:, nloc:], 0.0)
                # =================== layer 0 ===================
                if stop_at <= 1:
                    return
                with nc.named_scope("l0_tables"):
                    build_tables0()
                if stop_at <= 2:
                    early_out(xwT[0:P, 0:c_out])
                    return
                with nc.named_scope("l0_allgather"):
                    barrier()
                    nc.gpsimd.collective_compute(
                        "AllGather", mybir.AluOpType.bypass,
                        replica_groups=[core_ids],
                        ins=[tbl_loc[0][:]], outs=[tbl_full[0][:]])
                    barrier()
                if stop_at <= 3:
                    gdbg = wp.tile([P, c_out], BF16, tag="gdbg")
                    nc.sync.dma_start(out=gdbg[:], in_=tbl_full[0][0:P, 0:c_out])
                    gdbgf = wp.tile([P, c_out], F32, tag="gdbgf")
                    nc.vector.tensor_copy(gdbgf[:], gdbg[:])
                    early_out(gdbgf[:])
                    return

                # agg(l0) with interleaved post + layer-1 tables per sg
                def tail0(blocks):
                    s0 = blocks[0] * P
                    s1 = min(blocks[-1] * P + P, nblk_pad)
                    post01_slice(0, hT, s0, s1)
                    tables_slice(hT, w1_sb, h2, h2, xwT, tbl_loc[1],
                                 [(0, h, 0), (h, h2, 1)], blocks)

                with nc.named_scope("l0_agg"):
                    aggregate(tbl_full[0], h2, last=False, tail=tail0)
                if stop_at <= 5:
                    hdbg = wp.tile([P, c_out], F32, tag="hdbg")
                    nc.vector.tensor_copy(hdbg[:], hT[0:P, 0:c_out])
                    early_out(hdbg[:])
                    return

                # =================== layer 1 ===================
                with nc.named_scope("l1_allgather"):
                    barrier()
                    nc.gpsimd.collective_compute(
                        "AllGather", mybir.AluOpType.bypass,
                        replica_groups=[core_ids],
                        ins=[tbl_loc[1][:]], outs=[tbl_full[1][:]])
                    barrier()

                def tail1(blocks):
                    s0 = blocks[0] * P
                    s1 = min(blocks[-1] * P + P, nblk_pad)
                    post01_slice(1, h2T, s0, s1)
                    tables_slice(h2T, wl_sb, 128, c_out, xwTL, tblL_loc,
                                 [(0, 128, 2)], blocks)

                with nc.named_scope("l1_agg"):
                    aggregate(tbl_full[1], h2, last=False, tail=tail1)
                if stop_at <= 8:
                    hdbg2 = wp.tile([P, c_out], F32, tag="hdbg")
                    nc.vector.tensor_copy(hdbg2[:], h2T[0:P, 0:c_out])
                    early_out(hdbg2[:])
                    return

                # =================== last layer ===================
                with nc.named_scope("l2_allgather"):
                    barrier()
                    nc.gpsimd.collective_compute(
                        "AllGather", mybir.AluOpType.bypass,
                        replica_groups=[core_ids],
                        ins=[tblL_loc[:]], outs=[tblL_full[:]])
                    barrier()
                with nc.named_scope("l2_agg"):
                    aggregate(tblL_full, 128, last=True, tail=final_slice)

            _phases()

    nc.compile()
    return nc


# ----------------------------------------------------------------------------
# driver
# ----------------------------------------------------------------------------

_CACHE = {}
_RUNNER = {}


def _build_runner(nc, n_cores):
    """Persistent jitted executor (no donation; inputs stay on device)."""
    import jax
    from jax.sharding import Mesh, PartitionSpec
    from jax.experimental.shard_map import shard_map
    import concourse.mybir as mybir_
    from concourse import bass2jax
    from concourse.bass2jax import _bass_exec_p, partition_id_tensor

    bass2jax.install_neuronx_cc_hook()
    partition_name = (nc.partition_id_tensor.name
                      if nc.partition_id_tensor else None)
    in_names, out_names, out_avals, zero_outs = [], [], [], []
    for alloc in nc.m.functions[0].allocations:
        if not isinstance(alloc, mybir_.MemoryLocationSet):
            continue
        name = alloc.memorylocations[0].name
        if alloc.kind == "ExternalInput":
            if name != partition_name:
                in_names.append(name)
        elif alloc.kind == "ExternalOutput":
            out_names.append(name)
            shape = tuple(alloc.tensor_shape)
            dtype = mybir_.dt.np(alloc.dtype)
            out_avals.append(jax.core.ShapedArray(shape, dtype))
            zero_outs.append(np.zeros(shape, dtype))
    n_params = len(in_names)
    all_names = in_names + out_names
    if partition_name is not None:
        all_names.append(partition_name)

    def _body(*args):
        operands = list(args)
        if partition_name is not None:
            operands.append(partition_id_tensor())
        return tuple(_bass_exec_p.bind(
            *operands, out_avals=tuple(out_avals), in_names=tuple(all_names),
            out_names=tuple(out_names), lowering_input_output_aliases=(),
            sim_require_finite=True, sim_require_nnan=True, nc=nc))

    devices = jax.devices()[:n_cores]
    mesh = Mesh(np.asarray(devices), ("core",))
    n_out = len(out_names)
    fn = jax.jit(shard_map(_body, mesh=mesh,
                           in_specs=(PartitionSpec("core"),) * (n_params + n_out),
                           out_specs=(PartitionSpec("core"),) * n_out,
                           check_rep=False), keep_unused=True)
    return fn, in_names, out_names, out_avals, zero_outs, mesh


def _run_persistent(nc, in_maps, n_cores, key):
    import jax
    if key not in _RUNNER:
        fn, in_names, out_names, out_avals, zero_outs, mesh = \
            _build_runner(nc, n_cores)
        _RUNNER[key] = dict(fn=fn, in_names=in_names, out_names=out_names,
                            out_avals=out_avals, zero_outs=zero_outs,
                            mesh=mesh, dev_args=None)
    R = _RUNNER[key]
    concat_in = [np.concatenate([np.asarray(in_maps[c][nm])
                                 for c in range(n_cores)], axis=0)
                 for nm in R["in_names"]]
    concat_zero = [np.zeros((n_cores * z.shape[0], *z.shape[1:]), z.dtype)
                   for z in R["zero_outs"]]
    args = [jax.device_put(a) for a in concat_in + concat_zero]
    R["dev_args"] = args
    outs = R["fn"](*args)
    outs = [np.asarray(o) for o in outs]
    return {nm: outs[i].reshape(n_cores, *R["out_avals"][i].shape)
            for i, nm in enumerate(R["out_names"])}


# ---------------------------------------------------------------------------
# device timing: NTFF (neuron-profile) with difference-method fallback
# ---------------------------------------------------------------------------

def _axon_profile_hook(so_path="/opt/axon/libaxon_pjrt.so"):
    try:
        lib = ctypes.CDLL(so_path)
    except OSError:
        return None
    if not hasattr(lib, "axon_start_nrt_profile"):
        return None
    lib.axon_start_nrt_profile.argtypes = [ctypes.POINTER(ctypes.c_int64),
                                           ctypes.c_size_t]
    lib.axon_start_nrt_profile.restype = ctypes.c_int64
    lib.axon_stop_nrt_profile.argtypes = [ctypes.c_char_p]
    lib.axon_stop_nrt_profile.restype = ctypes.c_int64

    @contextlib.contextmanager
    def _hook(output_dir, device_ids):
        import jax
        jax.devices()
        if device_ids:
            ids = (ctypes.c_int64 * len(device_ids))(*device_ids)
            rc = lib.axon_start_nrt_profile(ids, len(device_ids))
        else:
            rc = lib.axon_start_nrt_profile(None, 0)
        if rc != 0:
            raise RuntimeError(f"axon_start_nrt_profile rc={rc}")
        try:
            yield
        finally:
            n = lib.axon_stop_nrt_profile(str(output_dir).encode())
            if n <= 0:
                print(f"profile capture wrote {n} files", file=sys.stderr)

    return _hook


def _ntff_exec_ns(key, devices=(0,)):
    """Profile one warm execution; return max NEFF device span in ns."""
    import jax
    hook = _axon_profile_hook()
    if hook is None:
        return None
    R = _RUNNER[key]
    fn, args = R["fn"], R["dev_args"]
    o = fn(*args)
    jax.block_until_ready(o)
    tmpdir = tempfile.mkdtemp(prefix="ntff_timing_")
    with hook(tmpdir, list(devices)):
        o = fn(*args)
        jax.block_until_ready(o)
    ntffs = sorted(glob.glob(os.path.join(tmpdir, "*_body*device*.ntff")))
    if not ntffs:
        return None
    times = []
    for i, ntff in enumerate(ntffs):
        # pair the ntff with its own executable's neff by name prefix
        prefix = os.path.basename(ntff).split("-device")[0]
        neffs = glob.glob(os.path.join(tmpdir, prefix + ".neff"))
        if not neffs:
            continue
        outj = os.path.join(tmpdir, f"prof_{i}.json")
        try:
            subprocess.run(
                ["neuron-profile", "view", "-n", neffs[0], "-s", ntff,
                 "--output-format=json", "--output-file", outj,
                 "--ignore-nc-buf-usage"],
                check=True, capture_output=True,
                env=dict(os.environ, NEURON_PROFILE_DBG_OUTPUT="2"))
            with open(outj) as f:
                j = json.load(f)
            times.append(float(j["summary"][0]["total_time"]) * 1e9)
        except Exception as e:
            print("neuron-profile failed:", e, file=sys.stderr)
    return max(times) if times else None


def time_device(inputs, iters=6, cfg=None):
    """Device execution time in ns.

    Primary: neuron-profile (NTFF) span of one warm on-device execution.
    Fallback: marginal per-execution wall time via the difference method."""
    import jax, time as _t
    cfg = cfg or FULL_CFG
    key = _ensure_built(cfg, inputs)
    if os.environ.get("BASS_TIME_MODE", "ntff") == "ntff":
        try:
            ns = _ntff_exec_ns(key)
        except Exception as e:
            print("ntff timing failed:", e, file=sys.stderr)
            ns = None
        if ns is not None:
            return ns
    R = _RUNNER[key]
    fn, args = R["fn"], R["dev_args"]
    o = fn(*args); jax.block_until_ready(o)
    t0 = _t.time()
    o = fn(*args); jax.block_until_ready(o)
    t1 = _t.time()
    base = t1 - t0
    t0 = _t.time()
    for _ in range(1 + iters):
        o = fn(*args)
    jax.block_until_ready(o)
    t1 = _t.time()
    per = (t1 - t0 - base) / iters
    return per * 1e9


def _ensure_built(cfg, inputs):
    weights = {k: np.asarray(inputs[k]) for k in
               ("W_st0", "b_st0", "W_ts0", "b_ts0", "W_st1", "b_st1",
                "W_ts1", "b_ts1", "W_last", "b_last")}
    lay, in_maps = host_prep(cfg, inputs["x"], inputs["edge_index"],
                             inputs["is_reversed"])
    wmap = host_prep_weights(cfg, **weights)
    for m in in_maps:
        m.update(wmap)
    sig = (lay.signature(), 99, "full")
    if sig not in _CACHE:
        _CACHE[sig] = build_program(cfg, lay, stop_at=99)
    nc = _CACHE[sig]
    key = str(sig)
    _run_persistent(nc, in_maps, cfg["n_cores"], key)
    return key


def run(cfg, x, edge_index, is_reversed, weights, use_sim=False, stop_at=99,
        agg_mode="full"):
    lay, in_maps = host_prep(cfg, x, edge_index, is_reversed)
    wmap = host_prep_weights(cfg, **weights)
    for m in in_maps:
        m.update(wmap)

    sig = (lay.signature(), stop_at, agg_mode)
    if sig in _CACHE:
        nc = _CACHE[sig]
    else:
        nc = build_program(cfg, lay, stop_at=stop_at)
        _CACHE[sig] = nc

    n_cores = cfg["n_cores"]
    if use_sim:
        import concourse.bass_interp as bass_interp
        sim = bass_interp.MultiCoreSim(nc, n_cores, require_finite=False,
                                       require_nnan=False)
        for c in range(n_cores):
            for k, v in in_maps[c].items():
                sim.cores[c].tensor(k)[:] = v
        sim.simulate()
        outs = [np.array(sim.cores[c].tensor("out")) for c in range(n_cores)]
    else:
        key = str(sig)
        res = _run_persistent(nc, in_maps, n_cores, key)
        outs = list(res["out"])
    return np.concatenate(outs, axis=0)


def kernel(x, edge_index, is_reversed, W_st0, b_st0, W_ts0, b_ts0,
           W_st1, b_st1, W_ts1, b_ts1, W_last, b_last):
    cfg = FULL_CFG
    weights = dict(W_st0=W_st0, b_st0=b_st0, W_ts0=W_ts0, b_ts0=b_ts0,
                   W_st1=W_st1, b_st1=b_st1, W_ts1=W_ts1, b_ts1=b_ts1,
                   W_last=W_last, b_last=b_last)
    out = run(cfg, x, edge_index, is_reversed, weights)
    return out.astype(np.float32)
